# revision 36
# baseline (speedup 1.0000x reference)
"""Trainium2 Bass kernel for nn_ACE15TEModel_41824391528638 (CFG sampling pipeline).

Reference semantics per decode step t over vocab V=155776:
  cfg = uncond + 2.0*(cond - uncond)
  mask tokens < AUDIO_START to -inf (EOS restored when t > MIN_TOKENS)
  top-k(64) filter, top-p(0.9) nucleus filter, temperature 0.85
  probs = softmax(filtered); token = categorical(key_t, filtered)  [Gumbel argmax]

Structural facts used:
  * Only tokens >= AUDIO_START plus EOS can survive the audio mask, i.e.
    L = V - AUDIO_START + 1 = 4108 "live" columns per step; every other
    output column is exactly 0 and every dead input column is never read.
  * The final keep set is {v : v >= sigma} where sigma is the top-p cutoff
    value, computable from the exact sorted top-64 values per row.
  * Gumbel noise of jax.random.categorical is input-independent -> precomputed
    host-side (bit-exact, jax CPU) and shipped as a kernel input.

Sharding: steps (T=512) data-parallel across 8 cores, 64 rows per core, one
row per SBUF partition.  Per core the kernel:
  - computes cfg for live columns (scalar_tensor_tensor)
  - exact top-64 per row via 8 rounds of DVE max8 + match_replace
  - exp-cumsum of sorted top-64 -> top-p cutoff sigma per row
  - keep mask = x >= sigma; softmax over kept; Gumbel argmax -> token index
  - writes only the live prob columns (output buffers are pre-zeroed by the
    runner: native run_bass_kernel_spmd pre-zeros ExternalOutput buffers,
    the PJRT path donates zero-filled buffers)
"""

import os
import sys

import numpy as np

for _p in (
    "/root/.axon_site",
    "/root/.axon_site/_ro/trn_rl_repo",
    "/root/.axon_site/_ro/pypackages",
    "/opt/trn_rl_repo",
):
    if os.path.isdir(_p) and _p not in sys.path:
        sys.path.append(_p)

import concourse.bass as bass
import concourse.mybir as mybir
from concourse import bass_utils
from concourse.tile import TileContext
from concourse.tile_rust import add_dep_helper

# Problem constants (hardcoded per spec)
T = 512
V = 155776
AUDIO_START = 151669
EOS = 151645
MIN_TOKENS = 1
CFG_SCALE = 2.0
TEMPERATURE = 0.85
TOP_P = 0.9
TOP_K = 64

N_CORES = 8
ROWS = T // N_CORES            # 64 steps per core
L = V - AUDIO_START + 1        # 4108 live columns: [EOS, AUDIO_START..V)
H = L // 2                     # 2054: row half width (two partitions per row)

NEG = -3.4028235e38            # float32 min (extraction sentinel)
EOS_KILL = -1.0e30             # finite stand-in for -inf EOS mask (avoids inf/nan)
BIG = 3.0e38
KILLC = 512.0                  # additive keep-mask bias: power of two, exact for
                               # {0,1} masks; exp((x-512-M)/temp) underflows to 0
INV_TEMP = 1.0 / TEMPERATURE
KB_C = KILLC * INV_TEMP        # folded into the exp bias so kept entries cancel
# 96-wide chunks over each half row (last takes the 134-col remainder).  The
# fast path pools each chunk's top-8; exact iff no chunk holds >8 of the
# row's top-64 (host-checked per input, safe builder otherwise).
CHUNKS = [(c * 96, (c + 1) * 96) for c in range(20)] + [(1920, H)]
# DMA/compute stripes (in chunks): fat stripes keep DMA descriptors >=1.9KB
# (descriptor rate, not bandwidth, limits thin transfers)
STRIPE_CHUNKS = [(0, 5), (5, 10), (10, 15), (15, 21)]
STRIPES = [(CHUNKS[lo][0], CHUNKS[hi - 1][1]) for lo, hi in STRIPE_CHUNKS]

F32 = mybir.dt.float32
U32 = mybir.dt.uint32

_CACHE: dict = {}


def _split_multi_waits(nc: bass.Bass) -> None:
    """This walrus build allows one semaphore wait per compute instruction.
    Hoist extra on_wait entries into standalone InstEventSemaphore ops placed
    just before the instruction on the same engine (engines are in-order, so
    semantics are identical)."""
    skip = (mybir.InstEventSemaphore,)
    for fn in nc.m.functions:
        for blk in fn.blocks:
            new_insts = []
            for inst in blk.instructions:
                si = getattr(inst, "sync_info", None)
                if (
                    si is not None
                    and si.on_wait is not None
                    and len(si.on_wait) > 1
                    and not isinstance(inst, skip)
                ):
                    for w in si.on_wait[:-1]:
                        ev = mybir.InstEventSemaphore(
                            name=f"{inst.name}-wsplit-{w.id}",
                            ins=[],
                            outs=[],
                            sync_info=mybir.SyncInfo(on_wait=[w], on_update=[]),
                        )
                        ev.engine = inst.engine
                        new_insts.append(ev)
                    inst.sync_info = mybir.SyncInfo(
                        on_wait=[si.on_wait[-1]], on_update=si.on_update
                    )
                new_insts.append(inst)
            blk.instructions = new_insts


def _split_wide_sem_clears(nc: bass.Bass, max_span: int = 16) -> None:
    """EVENT_SEMAPHORE_RANGE_CLEAR clears at most 16 semaphores per
    instruction on this toolchain; split wider ranges."""
    OPC = 176
    for fn in nc.m.functions:
        for blk in fn.blocks:
            new_insts = []
            for inst in blk.instructions:
                if (
                    isinstance(inst, mybir.InstISA)
                    and inst.isa_opcode == OPC
                    and inst.instr is not None
                    and len(inst.instr) > 14
                    and inst.instr[14] - inst.instr[13] + 1 > max_span
                ):
                    first, last = int(inst.instr[13]), int(inst.instr[14])
                    lo = first
                    while lo <= last:
                        hi = min(lo + max_span - 1, last)
                        words = list(inst.instr)
                        words[13], words[14] = lo, hi
                        ad = dict(inst.ant_dict)
                        ad["range_first"], ad["range_last"] = lo, hi
                        ni = mybir.InstISA(
                            name=f"{inst.name}-semclr-{lo}",
                            isa_opcode=inst.isa_opcode,
                            engine=inst.engine,
                            instr=words,
                            ant_dict=ad,
                            ins=[],
                            outs=[],
                        )
                        if lo == first and getattr(inst, "sync_info", None):
                            ni.sync_info = inst.sync_info
                        ni.ant_isa_is_sequencer_only = inst.ant_isa_is_sequencer_only
                        new_insts.append(ni)
                        lo = hi + 1
                else:
                    new_insts.append(inst)
            blk.instructions = new_insts


def _build_nc_safe() -> bass.Bass:
    """128-partition layout: step row r occupies partitions r (live cols
    [0:H), i.e. EOS + audio[0:H-1)) and r+64 (live cols [H:L)).  Per-half
    exact top-64 extraction, then the two halves' candidates are merged into
    both partition groups (redundant small-op compute beats cross-partition
    broadcast latency)."""
    nc = bass.Bass()
    xin_e = nc.declare_dram_parameter("xin", [2 * ROWS, 2 * H], F32, isOutput=False)
    gs_e = nc.declare_dram_parameter("gs", [2 * ROWS, H], F32, isOutput=False)
    probs_e = nc.declare_dram_parameter("probs", [ROWS, V], F32, isOutput=True)
    tok_e = nc.declare_dram_parameter("tokens", [ROWS, 1], F32, isOutput=True)

    K = TOP_K
    P = 2 * ROWS  # 128 partitions
    A = mybir.AluOpType
    X = mybir.AxisListType.X
    Exp = mybir.ActivationFunctionType.Exp
    Copy = mybir.ActivationFunctionType.Copy

    with TileContext(nc) as tc:
        with tc.tile_pool(name="pool", bufs=1) as pool:
            xin = pool.tile([P, 2 * H], F32)
            gs = pool.tile([P, H], F32)
            x = pool.tile([P, H], F32)
            w = pool.tile([P, H], F32)
            mask = pool.tile([P, H], F32)
            xm = pool.tile([P, H], F32)
            y = pool.tile([P, H], F32)
            yq = pool.tile([P, H], F32)
            kbias = pool.tile([P, H], F32)
            yqm = pool.tile([P, H], F32)

            s2 = pool.tile([P, K], F32)       # per-half top-64, sorted desc
            scand = pool.tile([P, 2 * K], F32)  # both halves' candidates
            w2 = pool.tile([P, 2 * K], F32)
            s = pool.tile([P, K], F32)        # row top-64 (same in both groups)
            e64 = pool.tile([P, K], F32)
            cum = pool.tile([P, K], F32)
            gx = pool.tile([P, K], F32)
            km = pool.tile([P, K], F32)
            t64 = pool.tile([P, K], F32)
            sk = pool.tile([P, K], F32)
            ms = pool.tile([P, K], F32)
            zeros64 = pool.tile([P, K], F32)

            negm = pool.tile([P, 1], F32)
            negmtk = pool.tile([P, 1], F32)
            z64 = pool.tile([P, 1], F32)
            thr = pool.tile([P, 1], F32)
            sig = pool.tile([P, 1], F32)
            zf = pool.tile([P, 1], F32)
            zfo = pool.tile([P, 1], F32)
            zfr = pool.tile([P, 1], F32)
            rzf = pool.tile([P, 1], F32)
            tm8 = pool.tile([P, 8], F32)
            tidx = pool.tile([P, 8], U32)
            tvb = pool.tile([ROWS, 1], F32)
            tib = pool.tile([ROWS, 1], U32)
            tfa = pool.tile([ROWS, 1], F32)
            tfb = pool.tile([ROWS, 1], F32)
            wbf = pool.tile([ROWS, 1], F32)
            td = pool.tile([ROWS, 1], F32)
            tokf = pool.tile([ROWS, 1], F32)

            HH = H // 2
            # Loads: cond/unc quarters on separate queues so x can start early
            nc.sync.dma_start(out=xin[:, 0:HH], in_=xin_e[:, 0:HH])
            nc.sync.dma_start(out=xin[:, H : H + HH], in_=xin_e[:, H : H + HH])
            nc.sync.dma_start(out=xin[:, HH:H], in_=xin_e[:, HH:H])
            nc.sync.dma_start(out=xin[:, H + HH : 2 * H], in_=xin_e[:, H + HH : 2 * H])
            nc.sync.dma_start(out=gs[:, 0:HH], in_=gs_e[:, 0:HH])
            nc.sync.dma_start(out=gs[:, HH:H], in_=gs_e[:, HH:H])

            nc.gpsimd.memset(zeros64[:, :], 0.0)

            # x = 2*cond - uncond, in two column stripes
            for c0, c1 in ((0, HH), (HH, H)):
                nc.vector.scalar_tensor_tensor(
                    out=x[:, c0:c1], in0=xin[:, c0:c1], scalar=float(CFG_SCALE),
                    in1=xin[:, H + c0 : H + c1], op0=A.mult, op1=A.subtract,
                )

            # tokens numerator (off the DVE critical path)
            nc.gpsimd.tensor_add(yq[:, :], x[:, :], gs[:, :])

            # per-half exact top-64: 8 rounds of max8 + match_replace
            nc.vector.max(out=s2[:, 0:8], in_=x[:, :])
            nc.vector.match_replace(
                out=w[:, :], in_to_replace=s2[:, 0:8], in_values=x[:, :],
                imm_value=NEG,
            )
            for r in range(1, K // 8):
                sl = s2[:, r * 8 : (r + 1) * 8]
                nc.vector.max(out=sl, in_=w[:, :])
                nc.vector.match_replace(
                    out=w[:, :], in_to_replace=sl, in_values=w[:, :], imm_value=NEG
                )

            # merge the two halves' top-64 into both partition groups
            nc.vector.tensor_copy(scand[:, 0:K], s2[:, :])
            nc.sync.dma_start(out=scand[0:ROWS, K : 2 * K], in_=s2[ROWS:P, :])
            nc.sync.dma_start(out=scand[ROWS:P, K : 2 * K], in_=s2[0:ROWS, :])

            # row top-64 from the 128 candidates
            nc.vector.max(out=s[:, 0:8], in_=scand[:, :])
            nc.vector.match_replace(
                out=w2[:, :], in_to_replace=s[:, 0:8], in_values=scand[:, :],
                imm_value=NEG,
            )
            for r in range(1, K // 8):
                sl = s[:, r * 8 : (r + 1) * 8]
                nc.vector.max(out=sl, in_=w2[:, :])
                nc.vector.match_replace(
                    out=w2[:, :], in_to_replace=sl, in_values=w2[:, :], imm_value=NEG
                )

            # M = s[:,0];  exp biases
            nc.scalar.mul(negm[:, :], s[:, 0:1], -1.0)
            nc.scalar.activation(
                negmtk[:, :], s[:, 0:1], Copy, bias=-KB_C, scale=-INV_TEMP
            )

            # E = exp(s - M), Z64, exclusive cumsum -> top-p cutoff sigma
            nc.scalar.activation(
                e64[:, :], s[:, :], Exp, bias=negm[:, :], scale=1.0,
                accum_out=z64[:, :],
            )
            nc.vector.tensor_tensor_scan(
                out=cum[:, :], data0=e64[:, :], data1=zeros64[:, :],
                initial=0.0, op0=A.add, op1=A.add,
            )
            nc.vector.tensor_sub(gx[:, :], cum[:, :], e64[:, :])
            nc.vector.tensor_scalar(
                out=thr[:, :], in0=z64[:, :], scalar1=float(TOP_P), scalar2=None,
                op0=A.mult,
            )
            nc.vector.tensor_scalar(
                out=km[:, :], in0=gx[:, :], scalar1=thr[:, :], scalar2=None,
                op0=A.is_le,
            )
            nc.vector.tensor_scalar(
                out=t64[:, :], in0=km[:, :], scalar1=-BIG, scalar2=BIG,
                op0=A.mult, op1=A.add,
            )
            nc.vector.tensor_mul(sk[:, :], s[:, :], km[:, :])
            nc.vector.tensor_add(ms[:, :], sk[:, :], t64[:, :])
            nc.vector.tensor_reduce(sig[:, :], ms[:, :], X, A.min)

            # keep mask;  probs path: xm = x + KILLC*mask, exp underflow kills
            nc.vector.tensor_scalar(
                out=mask[:, :], in0=x[:, :], scalar1=sig[:, :], scalar2=None,
                op0=A.is_ge,
            )
            nc.vector.scalar_tensor_tensor(
                out=xm[:, :], in0=mask[:, :], scalar=KILLC, in1=x[:, :],
                op0=A.mult, op1=A.add,
            )
            nc.scalar.activation(
                y[:, :], xm[:, :], Exp, bias=negmtk[:, :], scale=INV_TEMP,
                accum_out=zf[:, :],
            )
            # Zf = sum over both halves: swap-merge across partition groups
            nc.sync.dma_start(out=zfo[0:ROWS, :], in_=zf[ROWS:P, :])
            nc.sync.dma_start(out=zfo[ROWS:P, :], in_=zf[0:ROWS, :])
            nc.vector.tensor_add(zfr[:, :], zf[:, :], zfo[:, :])
            nc.vector.reciprocal(rzf[:, :], zfr[:, :])
            nc.vector.tensor_scalar_mul(y[:, :], y[:, :], rzf[:, :])

            # tokens: yqm = yq + (mask-1)*KILLC, exact for kept entries
            nc.vector.tensor_scalar(
                out=kbias[:, :], in0=mask[:, :], scalar1=KILLC, scalar2=-KILLC,
                op0=A.mult, op1=A.add,
            )
            nc.vector.tensor_add(yqm[:, :], yq[:, :], kbias[:, :])
            nc.vector.max(out=tm8[:, :], in_=yqm[:, :])
            nc.vector.max_index(out=tidx[:, :], in_max=tm8[:, :], in_values=yqm[:, :])

            # winner across halves (B strictly greater -> B, ties -> A)
            nc.sync.dma_start(out=tvb[:, :], in_=tm8[ROWS:P, 0:1])
            nc.sync.dma_start(out=tib[:, :], in_=tidx[ROWS:P, 0:1])
            nc.vector.tensor_tensor(
                out=wbf[:, :], in0=tvb[:, :], in1=tm8[0:ROWS, 0:1], op=A.is_gt
            )
            nc.vector.tensor_copy(tfa[:, :], tidx[0:ROWS, 0:1])
            nc.vector.tensor_copy(tfb[:, :], tib[:, :])
            nc.vector.tensor_scalar_add(tfb[:, :], tfb[:, :], float(H))
            nc.vector.tensor_sub(td[:, :], tfb[:, :], tfa[:, :])
            nc.vector.tensor_mul(td[:, :], td[:, :], wbf[:, :])
            nc.vector.tensor_add(tokf[:, :], tfa[:, :], td[:, :])

            # Stores: only live prob columns; everything else stays zero
            nc.sync.dma_start(out=probs_e[:, EOS : EOS + 1], in_=y[0:ROWS, 0:1])
            nc.sync.dma_start(
                out=probs_e[:, AUDIO_START : AUDIO_START + H - 1],
                in_=y[0:ROWS, 1:H],
            )
            nc.sync.dma_start(
                out=probs_e[:, AUDIO_START + H - 1 : V], in_=y[ROWS:P, :]
            )
            nc.sync.dma_start(out=tok_e[:, :], in_=tokf[:, :])

    _split_multi_waits(nc)
    return nc


def _build_nc_fast() -> bass.Bass:
    """Fast extraction variant: per-chunk top-8 pooling (values only) replaces
    the 8-round full-width extraction.  Exact when no chunk holds more than 8
    of its row's top-64 -- guaranteed by the host-side input check, which
    otherwise selects the safe builder.  Input DMA and cfg are striped so the
    chunk maxes start while later stripes are still loading; the keep-mask
    exp runs on GpSimd/ScalarE in parallel with the token argmax on DVE."""
    nc = bass.Bass()
    P = 2 * ROWS  # 128 partitions
    xin_e = nc.declare_dram_parameter("xin", [P * 2 * H], F32, isOutput=False)
    gs_e = nc.declare_dram_parameter("gs", [P * H], F32, isOutput=False)
    probs_e = nc.declare_dram_parameter("probs", [ROWS, V], F32, isOutput=True)
    tok_e = nc.declare_dram_parameter("tokens", [ROWS, 1], F32, isOutput=True)

    K = TOP_K
    NCH = len(CHUNKS)             # 32 chunks of the half row
    PW = 8 * NCH                  # 256: pooled candidates per half
    A = mybir.AluOpType
    X = mybir.AxisListType.X
    Exp = mybir.ActivationFunctionType.Exp
    Copy = mybir.ActivationFunctionType.Copy

    with TileContext(nc) as tc:
        with tc.tile_pool(name="pool", bufs=1) as pool:
            xin = pool.tile([P, 2 * H], F32)
            gs = pool.tile([P, H], F32)
            x = pool.tile([P, H], F32)
            xm = pool.tile([P, H], F32)
            y = pool.tile([P, H], F32)
            yq = pool.tile([P, H], F32)
            kbias = pool.tile([P, H], F32)
            yqm = pool.tile([P, H], F32)

            scand = pool.tile([P, 2 * PW], F32)  # both halves' chunk top-8 pools
            w2 = pool.tile([P, 2 * PW], F32)
            s = pool.tile([P, K], F32)           # row top-64, sorted desc
            e64 = pool.tile([P, K], F32)
            cum = pool.tile([P, K], F32)
            gx = pool.tile([P, K], F32)
            km = pool.tile([P, K], F32)
            t64 = pool.tile([P, K], F32)
            sk = pool.tile([P, K], F32)
            ms = pool.tile([P, K], F32)
            zeros64 = pool.tile([P, K], F32)

            negm = pool.tile([P, 1], F32)
            negmtk = pool.tile([P, 1], F32)
            z64 = pool.tile([P, 1], F32)
            thr = pool.tile([P, 1], F32)
            sig = pool.tile([P, 1], F32)
            zf = pool.tile([P, 1], F32)
            zfo = pool.tile([P, 1], F32)
            zfr = pool.tile([P, 1], F32)
            lnz = pool.tile([P, 1], F32)
            nlnz = pool.tile([P, 1], F32)
            bias2 = pool.tile([P, 1], F32)
            tm8 = pool.tile([P, 8], F32)
            tidx = pool.tile([P, 8], U32)
            tvb = pool.tile([ROWS, 1], F32)
            tib = pool.tile([ROWS, 1], U32)
            tfa = pool.tile([ROWS, 1], F32)
            tfb = pool.tile([ROWS, 1], F32)
            wbf = pool.tile([ROWS, 1], F32)
            td = pool.tile([ROWS, 1], F32)
            tokf = pool.tile([ROWS, 1], F32)

            # stripe loads from host-packed contiguous blocks (cond+unc pairs
            # first-needed-first): fully-linear DRAM reads, fat descriptors
            off = 0
            for si, (a, b) in enumerate(STRIPES):
                w = b - a
                for col0 in (a, H + a):
                    blk = xin_e[off : off + P * w].rearrange("(p c) -> p c", p=P)
                    nc.sync.dma_start(out=xin[:, col0 : col0 + w], in_=blk)
                    off += P * w
            gw = H // 2
            for gi, col0 in enumerate((0, gw)):
                blk = gs_e[gi * P * gw : (gi + 1) * P * gw].rearrange(
                    "(p c) -> p c", p=P
                )
                nc.gpsimd.dma_start(out=gs[:, col0 : col0 + gw], in_=blk)

            nc.gpsimd.memset(zeros64[:, :], 0.0)

            # cfg + chunk top-8s, stripe by stripe
            for si, (a, b) in enumerate(STRIPES):
                nc.vector.scalar_tensor_tensor(
                    out=x[:, a:b], in0=xin[:, a:b], scalar=float(CFG_SCALE),
                    in1=xin[:, H + a : H + b], op0=A.mult, op1=A.subtract,
                )
                for ci in range(*STRIPE_CHUNKS[si]):
                    ca, cb = CHUNKS[ci]
                    nc.vector.max(
                        out=scand[:, 8 * ci : 8 * ci + 8], in_=x[:, ca:cb]
                    )

            # tokens numerator off the DVE critical path
            nc.gpsimd.tensor_add(yq[:, :], x[:, :], gs[:, :])

            # merge both halves' pools into both partition groups
            nc.sync.dma_start(
                out=scand[0:ROWS, PW : 2 * PW], in_=scand[ROWS:P, 0:PW]
            )
            nc.sync.dma_start(
                out=scand[ROWS:P, PW : 2 * PW], in_=scand[0:ROWS, 0:PW]
            )

            # row top-64 from the 512 pooled candidates
            nc.vector.max(out=s[:, 0:8], in_=scand[:, :])
            nc.vector.match_replace(
                out=w2[:, :], in_to_replace=s[:, 0:8], in_values=scand[:, :],
                imm_value=NEG,
            )
            for r in range(1, K // 8):
                sl = s[:, r * 8 : (r + 1) * 8]
                nc.vector.max(out=sl, in_=w2[:, :])
                nc.vector.match_replace(
                    out=w2[:, :], in_to_replace=sl, in_values=w2[:, :], imm_value=NEG
                )

            # M = s[:,0];  exp biases
            nc.scalar.mul(negm[:, :], s[:, 0:1], -1.0)
            nc.scalar.mul(negmtk[:, :], s[:, 0:1], -INV_TEMP)

            # E = exp(s - M), Z64, exclusive cumsum -> top-p cutoff sigma
            nc.scalar.activation(
                e64[:, :], s[:, :], Exp, bias=negm[:, :], scale=1.0,
                accum_out=z64[:, :],
            )
            nc.vector.tensor_tensor_scan(
                out=cum[:, :], data0=e64[:, :], data1=zeros64[:, :],
                initial=0.0, op0=A.add, op1=A.add,
            )
            nc.vector.tensor_sub(gx[:, :], cum[:, :], e64[:, :])
            nc.vector.tensor_scalar(
                out=thr[:, :], in0=z64[:, :], scalar1=float(TOP_P), scalar2=None,
                op0=A.mult,
            )
            nc.vector.tensor_scalar(
                out=km[:, :], in0=gx[:, :], scalar1=thr[:, :], scalar2=None,
                op0=A.is_le,
            )
            nc.vector.tensor_scalar(
                out=t64[:, :], in0=km[:, :], scalar1=-BIG, scalar2=BIG,
                op0=A.mult, op1=A.add,
            )
            nc.vector.tensor_mul(sk[:, :], s[:, :], km[:, :])
            nc.vector.tensor_add(ms[:, :], sk[:, :], t64[:, :])
            nc.vector.tensor_reduce(sig[:, :], ms[:, :], X, A.min)

            # fused keep mask -> exact additive kill bias (0 kept, -512 dropped)
            nc.vector.tensor_scalar(
                out=kbias[:, :], in0=x[:, :], scalar1=sig[:, :], scalar2=-KILLC,
                op0=A.is_lt, op1=A.mult,
            )
            nc.vector.tensor_add(xm[:, :], x[:, :], kbias[:, :])
            # probs = exp((xm - M)/temp)/Zf; dropped entries underflow to 0 and
            # the ACT accumulator therefore yields the masked Zf directly
            nc.scalar.activation(
                y[:, :], xm[:, :], Exp, bias=negmtk[:, :], scale=INV_TEMP,
                accum_out=zf[:, :],
            )
            nc.sync.dma_start(out=zfo[0:ROWS, :], in_=zf[ROWS:P, :])
            nc.sync.dma_start(out=zfo[ROWS:P, :], in_=zf[0:ROWS, :])
            # normalization entirely on ScalarE: P = exp((xm-M)/T - ln Zf);
            # DVE runs the token argmax in parallel, stores chase only ACT
            nc.scalar.add(zfr[:, :], zf[:, :], zfo[:, :])
            nc.scalar.activation(
                lnz[:, :], zfr[:, :], mybir.ActivationFunctionType.Ln
            )
            nc.scalar.mul(nlnz[:, :], lnz[:, :], -1.0)
            nc.scalar.add(bias2[:, :], negmtk[:, :], nlnz[:, :])
            nc.scalar.activation(
                y[:, :], xm[:, :], Exp, bias=bias2[:, :], scale=INV_TEMP
            )
            nc.sync.dma_start(out=probs_e[:, EOS : EOS + 1], in_=y[0:ROWS, 0:1])
            RH = ROWS // 2
            for q in range(2):
                r0, r1 = q * RH, (q + 1) * RH
                nc.sync.dma_start(
                    out=probs_e[r0:r1, AUDIO_START : AUDIO_START + H - 1],
                    in_=y[r0:r1, 1:H],
                )
                nc.sync.dma_start(
                    out=probs_e[r0:r1, AUDIO_START + H - 1 : V],
                    in_=y[ROWS + r0 : ROWS + r1, :],
                )

            nc.vector.tensor_add(yqm[:, :], yq[:, :], kbias[:, :])
            nc.vector.max(out=tm8[:, :], in_=yqm[:, :])
            nc.vector.max_index(out=tidx[:, :], in_max=tm8[:, :], in_values=yqm[:, :])

            # winner across halves (B strictly greater -> B, ties -> A)
            nc.sync.dma_start(out=tvb[:, :], in_=tm8[ROWS:P, 0:1])
            nc.sync.dma_start(out=tib[:, :], in_=tidx[ROWS:P, 0:1])
            nc.vector.tensor_tensor(
                out=wbf[:, :], in0=tvb[:, :], in1=tm8[0:ROWS, 0:1], op=A.is_gt
            )
            nc.vector.tensor_copy(tfa[:, :], tidx[0:ROWS, 0:1])
            nc.vector.tensor_copy(tfb[:, :], tib[:, :])
            nc.vector.tensor_scalar_add(tfb[:, :], tfb[:, :], float(H))
            nc.vector.tensor_sub(td[:, :], tfb[:, :], tfa[:, :])
            nc.vector.tensor_mul(td[:, :], td[:, :], wbf[:, :])
            nc.vector.tensor_add(tokf[:, :], tfa[:, :], td[:, :])

            nc.sync.dma_start(out=tok_e[:, :], in_=tokf[:, :])

    _split_multi_waits(nc)
    return nc


def fast_path_ok(live: np.ndarray) -> bool:
    """True iff per-chunk top-8 pooling recovers every row's exact top-64,
    i.e. no 64-wide chunk of either half holds more than 8 values >= the
    row's 64th-largest (ties counted conservatively)."""
    x = (live[:, 0] * np.float32(2.0)) - live[:, 1]
    tau = np.partition(x, L - TOP_K, axis=1)[:, L - TOP_K]
    ge = x >= tau[:, None]
    if not np.all(ge.sum(axis=1) == TOP_K):
        return False  # ties at the boundary: let the safe path handle them
    for half in range(2):
        g = ge[:, half * H : (half + 1) * H]
        for a, b in CHUNKS:
            if g[:, a:b].sum(axis=1).max() > 8:
                return False
    return True


def _get_nc(fast: bool) -> bass.Bass:
    key = "nc_fast" if fast else "nc_safe"
    if key not in _CACHE:
        _CACHE[key] = _build_nc_fast() if fast else _build_nc_safe()
    return _CACHE[key]


def _gumbel_live_scaled() -> np.ndarray:
    """temp * gumbel noise of jax.random.categorical at the live columns,
    bit-exact vs the reference (jax threefry on CPU), [T, L] float32."""
    if "gs" in _CACHE:
        return _CACHE["gs"]
    import jax
    import jax.numpy as jnp

    with jax.default_device(jax.devices("cpu")[0]):
        keys = jax.random.split(jax.random.key(1), T)
        gum = jax.jit(
            jax.vmap(lambda k: jax.random.gumbel(k, (V,), jnp.float32))
        )(keys)
        gum = np.asarray(gum)
    gl = np.empty((T, L), np.float32)
    gl[:, 0] = gum[:, EOS]
    gl[:, 1:] = gum[:, AUDIO_START:]
    gl *= np.float32(TEMPERATURE)
    _CACHE["gs"] = gl
    return gl


def make_live(logits: np.ndarray) -> np.ndarray:
    logits = np.asarray(logits, dtype=np.float32)
    live = np.empty((T, 2, L), np.float32)
    live[:, :, 0] = logits[:, :, EOS]
    live[:, :, 1:] = logits[:, :, AUDIO_START:]
    # EOS kill for steps <= MIN_TOKENS: force cond so 2*cond - uncond ~ -1e30,
    # far below any live logit -> never in the top-64, prob exactly 0.
    live[: MIN_TOKENS + 1, 0, 0] = EOS_KILL / 2
    return live


def make_in_maps(live: np.ndarray, fast: bool) -> list[dict[str, np.ndarray]]:
    gs = _gumbel_live_scaled()
    maps = []
    for c in range(N_CORES):
        r0, r1 = c * ROWS, (c + 1) * ROWS
        # partition p<64: row p's live cols [0:H); p>=64: row p-64's [H:L)
        xin2 = np.empty((2 * ROWS, 2 * H), np.float32)
        xin2[:ROWS, 0:H] = live[r0:r1, 0, 0:H]
        xin2[:ROWS, H:] = live[r0:r1, 1, 0:H]
        xin2[ROWS:, 0:H] = live[r0:r1, 0, H:L]
        xin2[ROWS:, H:] = live[r0:r1, 1, H:L]
        gs2 = np.empty((2 * ROWS, H), np.float32)
        gs2[:ROWS] = gs[r0:r1, 0:H]
        gs2[ROWS:] = gs[r0:r1, H:L]
        if fast:
            # stripe-contiguous packing mirroring the fast builder's reads
            blocks = []
            for a, b in STRIPES:
                blocks.append(xin2[:, a:b].ravel())
                blocks.append(xin2[:, H + a : H + b].ravel())
            xinp = np.concatenate(blocks)
            gw = H // 2
            gsp = np.concatenate(
                [gs2[:, 0:gw].ravel(), gs2[:, gw:H].ravel()]
            )
            maps.append({"xin": xinp, "gs": gsp})
        else:
            maps.append({"xin": xin2, "gs": gs2})
    return maps


def postprocess(results: list[dict[str, np.ndarray]]):
    probs = np.concatenate([r["probs"] for r in results], axis=0)
    tidx = np.concatenate(
        [r["tokens"][:, 0].astype(np.int64) for r in results], axis=0
    )
    tokens = np.where(tidx == 0, EOS, AUDIO_START - 1 + tidx).astype(np.int32)
    return tokens, probs


def kernel(logits: np.ndarray):
    live = make_live(logits)
    fast = os.environ.get("SAMPLER_FORCE_PATH", "")
    use_fast = fast_path_ok(live) if fast == "" else (fast == "fast")
    nc = _get_nc(use_fast)
    in_maps = make_in_maps(live, use_fast)
    res = bass_utils.run_bass_kernel_spmd(
        nc, in_maps, core_ids=list(range(N_CORES))
    )
    _CACHE["last_run"] = res
    return postprocess(res.results)


# revision 40
# speedup vs baseline: 1.5151x; 1.5151x over previous
"""Trainium2 Bass kernel for nn_ACE15TEModel_41824391528638 (CFG sampling).

Reference semantics per decode step t over vocab V=155776:
  cfg = uncond + 2.0*(cond - uncond)
  mask tokens < AUDIO_START to -inf (EOS restored when t > MIN_TOKENS)
  top-k(64) filter, top-p(0.9) nucleus filter, temperature 0.85
  probs = softmax(filtered); token = categorical(key_t, filtered) [Gumbel argmax]

Structure exploited:
  * Only the L = V-AUDIO_START+1 = 4108 "live" columns (EOS + audio tokens)
    can survive the audio mask; every other prob column is exactly 0 and no
    dead input column is ever read.  The runner pre-zeros ExternalOutput
    buffers (native path zero-fills, PJRT path donates zero buffers), so the
    kernel writes only the live columns of the full [T, V] probs output.
  * The final keep set is {v : v >= sigma}, sigma derived from the exact
    sorted top-64 values per row via the exp-cumsum crossing of top-p.
  * The Gumbel noise of jax.random.categorical is input-independent ->
    precomputed bit-exactly on host (jax CPU threefry) as a kernel input.

Sharding: steps (T=512) data-parallel over 8 cores; per core 64 steps, each
row split into two 2054-wide halves on SBUF partitions r and r+64.

Fast path (used whenever the host check passes, incl. the spec's input):
  striped DMA + cfg -> per-96-col-chunk top-8 pools (DVE max8) -> cross-
  partition pool merge -> exact row top-64 via 8 rounds of max8+match_replace
  on the 336-wide pool -> top-p cutoff sigma -> additive -512 kill bias ->
  probs = exp((x-M)/T - lnZf) on ScalarE (stores chase only ACT) while DVE
  runs the Gumbel argmax (max8 + max_index) -> merged winner across halves.
  Exactness condition (no chunk holds >8 of its row's top-64) is verified on
  host against the actual input; otherwise the safe builder (full-width
  8-round extraction per half) is selected, which is exact unconditionally.
"""

import os
import sys

import numpy as np

for _p in (
    "/root/.axon_site",
    "/root/.axon_site/_ro/trn_rl_repo",
    "/root/.axon_site/_ro/pypackages",
    "/opt/trn_rl_repo",
):
    if os.path.isdir(_p) and _p not in sys.path:
        sys.path.append(_p)

import concourse.bass as bass
import concourse.mybir as mybir
from concourse import bass_utils
from concourse.tile import TileContext

# Problem constants (hardcoded per spec)
T = 512
V = 155776
AUDIO_START = 151669
EOS = 151645
MIN_TOKENS = 1
CFG_SCALE = 2.0
TEMPERATURE = 0.85
TOP_P = 0.9
TOP_K = 64

N_CORES = 8
ROWS = T // N_CORES            # 64 steps per core
L = V - AUDIO_START + 1        # 4108 live columns: [EOS, AUDIO_START..V)
H = L // 2                     # 2054: row half width (two partitions per row)

NEG = -3.4028235e38            # float32 min (extraction sentinel)
EOS_KILL = -1.0e30             # finite stand-in for -inf EOS mask (avoids inf/nan)
BIG = 3.0e38
KILLC = 512.0                  # additive keep-mask bias: power of two, exact for
                               # {0,1} masks; exp((x-512-M)/temp) underflows to 0
INV_TEMP = 1.0 / TEMPERATURE
KB_C = KILLC * INV_TEMP        # folded into the exp bias so kept entries cancel
# 96-wide chunks over each half row (last takes the 134-col remainder).  The
# fast path pools each chunk's top-8; exact iff no chunk holds >8 of the
# row's top-64 (host-checked per input, safe builder otherwise).
CHUNKS = [(c * 96, (c + 1) * 96) for c in range(20)] + [(1920, H)]
# DMA/compute stripes (in chunks): fat stripes keep DMA descriptors >=2.6KB
# (descriptor rate, not bandwidth, limits thin transfers)
STRIPE_CHUNKS = [(0, 7), (7, 14), (14, 21)]
STRIPES = [(CHUNKS[lo][0], CHUNKS[hi - 1][1]) for lo, hi in STRIPE_CHUNKS]

F32 = mybir.dt.float32
U32 = mybir.dt.uint32

_CACHE: dict = {}


def _ensure_trace_hooks() -> None:
    """If the environment enables BASS_TRACE, run_bass_kernel_spmd imports
    antenv.axon_hooks, which some images lack; synthesize a compatible module
    (wired to the axon NTFF ctypes hook when available) so tracing works
    instead of crashing.  No-op when tracing is off or the module exists."""
    try:
        import antenv.axon_hooks  # noqa: F401
        return
    except ImportError:
        pass
    import types

    try:
        import antenv
    except ImportError:
        return
    mod = types.ModuleType("antenv.axon_hooks")
    holder: dict = {"hook": None, "tried": False}

    def set_axon_ntff_profile_hook(h):
        holder["hook"] = h

    def get_axon_ntff_profile_hook():
        if holder["hook"] is None and not holder["tried"]:
            holder["tried"] = True
            try:
                from trn_agent_boot.trn_boot import _ntff_profile_via_ctypes

                so = "/opt/axon/libaxon_pjrt.so"
                if os.path.exists(so):
                    holder["hook"] = _ntff_profile_via_ctypes(so)
            except Exception:
                pass
        return holder["hook"]

    mod.set_axon_ntff_profile_hook = set_axon_ntff_profile_hook
    mod.get_axon_ntff_profile_hook = get_axon_ntff_profile_hook
    sys.modules["antenv.axon_hooks"] = mod
    antenv.axon_hooks = mod


_ensure_trace_hooks()

# Artifact upload has no remote store in sandboxed runs; degrade locally.
_orig_upload = bass_utils.upload_artifacts


def _safe_upload(tmpdir: str) -> str:
    try:
        return _orig_upload(tmpdir)
    except Exception:
        return tmpdir


bass_utils.upload_artifacts = _safe_upload


def _split_multi_waits(nc: bass.Bass) -> None:
    """This walrus build allows one semaphore wait per compute instruction.
    Hoist extra on_wait entries into standalone InstEventSemaphore ops placed
    just before the instruction on the same engine (engines are in-order, so
    semantics are identical)."""
    skip = (mybir.InstEventSemaphore,)
    for fn in nc.m.functions:
        for blk in fn.blocks:
            new_insts = []
            for inst in blk.instructions:
                si = getattr(inst, "sync_info", None)
                if (
                    si is not None
                    and si.on_wait is not None
                    and len(si.on_wait) > 1
                    and not isinstance(inst, skip)
                ):
                    for w in si.on_wait[:-1]:
                        ev = mybir.InstEventSemaphore(
                            name=f"{inst.name}-wsplit-{w.id}",
                            ins=[],
                            outs=[],
                            sync_info=mybir.SyncInfo(on_wait=[w], on_update=[]),
                        )
                        ev.engine = inst.engine
                        new_insts.append(ev)
                    inst.sync_info = mybir.SyncInfo(
                        on_wait=[si.on_wait[-1]], on_update=si.on_update
                    )
                new_insts.append(inst)
            blk.instructions = new_insts


def _build_nc_safe() -> bass.Bass:
    """128-partition layout: step row r occupies partitions r (live cols
    [0:H), i.e. EOS + audio[0:H-1)) and r+64 (live cols [H:L)).  Per-half
    exact top-64 extraction, then the two halves' candidates are merged into
    both partition groups (redundant small-op compute beats cross-partition
    broadcast latency)."""
    nc = bass.Bass()
    xin_e = nc.declare_dram_parameter("xin", [2 * ROWS, 2 * H], F32, isOutput=False)
    gs_e = nc.declare_dram_parameter("gs", [2 * ROWS, H], F32, isOutput=False)
    probs_e = nc.declare_dram_parameter("probs", [ROWS, V], F32, isOutput=True)
    tok_e = nc.declare_dram_parameter("tokens", [ROWS, 1], F32, isOutput=True)

    K = TOP_K
    P = 2 * ROWS  # 128 partitions
    A = mybir.AluOpType
    X = mybir.AxisListType.X
    Exp = mybir.ActivationFunctionType.Exp
    Copy = mybir.ActivationFunctionType.Copy

    with TileContext(nc) as tc:
        with tc.tile_pool(name="pool", bufs=1) as pool:
            xin = pool.tile([P, 2 * H], F32)
            gs = pool.tile([P, H], F32)
            x = pool.tile([P, H], F32)
            w = pool.tile([P, H], F32)
            mask = pool.tile([P, H], F32)
            xm = pool.tile([P, H], F32)
            y = pool.tile([P, H], F32)
            yq = pool.tile([P, H], F32)
            kbias = pool.tile([P, H], F32)
            yqm = pool.tile([P, H], F32)

            s2 = pool.tile([P, K], F32)       # per-half top-64, sorted desc
            scand = pool.tile([P, 2 * K], F32)  # both halves' candidates
            w2 = pool.tile([P, 2 * K], F32)
            s = pool.tile([P, K], F32)        # row top-64 (same in both groups)
            e64 = pool.tile([P, K], F32)
            cum = pool.tile([P, K], F32)
            gx = pool.tile([P, K], F32)
            km = pool.tile([P, K], F32)
            t64 = pool.tile([P, K], F32)
            sk = pool.tile([P, K], F32)
            ms = pool.tile([P, K], F32)
            zeros64 = pool.tile([P, K], F32)

            negm = pool.tile([P, 1], F32)
            negmtk = pool.tile([P, 1], F32)
            z64 = pool.tile([P, 1], F32)
            thr = pool.tile([P, 1], F32)
            sig = pool.tile([P, 1], F32)
            zf = pool.tile([P, 1], F32)
            zfo = pool.tile([P, 1], F32)
            zfr = pool.tile([P, 1], F32)
            rzf = pool.tile([P, 1], F32)
            tm8 = pool.tile([P, 8], F32)
            tidx = pool.tile([P, 8], U32)
            tvb = pool.tile([ROWS, 1], F32)
            tib = pool.tile([ROWS, 1], U32)
            tfa = pool.tile([ROWS, 1], F32)
            tfb = pool.tile([ROWS, 1], F32)
            wbf = pool.tile([ROWS, 1], F32)
            td = pool.tile([ROWS, 1], F32)
            tokf = pool.tile([ROWS, 1], F32)

            HH = H // 2
            # Loads: cond/unc quarters on separate queues so x can start early
            nc.sync.dma_start(out=xin[:, 0:HH], in_=xin_e[:, 0:HH])
            nc.sync.dma_start(out=xin[:, H : H + HH], in_=xin_e[:, H : H + HH])
            nc.sync.dma_start(out=xin[:, HH:H], in_=xin_e[:, HH:H])
            nc.sync.dma_start(out=xin[:, H + HH : 2 * H], in_=xin_e[:, H + HH : 2 * H])
            nc.sync.dma_start(out=gs[:, 0:HH], in_=gs_e[:, 0:HH])
            nc.sync.dma_start(out=gs[:, HH:H], in_=gs_e[:, HH:H])

            nc.gpsimd.memset(zeros64[:, :], 0.0)

            # x = 2*cond - uncond, in two column stripes
            for c0, c1 in ((0, HH), (HH, H)):
                nc.vector.scalar_tensor_tensor(
                    out=x[:, c0:c1], in0=xin[:, c0:c1], scalar=float(CFG_SCALE),
                    in1=xin[:, H + c0 : H + c1], op0=A.mult, op1=A.subtract,
                )

            # tokens numerator (off the DVE critical path)
            nc.gpsimd.tensor_add(yq[:, :], x[:, :], gs[:, :])

            # per-half exact top-64: 8 rounds of max8 + match_replace
            nc.vector.max(out=s2[:, 0:8], in_=x[:, :])
            nc.vector.match_replace(
                out=w[:, :], in_to_replace=s2[:, 0:8], in_values=x[:, :],
                imm_value=NEG,
            )
            for r in range(1, K // 8):
                sl = s2[:, r * 8 : (r + 1) * 8]
                nc.vector.max(out=sl, in_=w[:, :])
                nc.vector.match_replace(
                    out=w[:, :], in_to_replace=sl, in_values=w[:, :], imm_value=NEG
                )

            # merge the two halves' top-64 into both partition groups
            nc.vector.tensor_copy(scand[:, 0:K], s2[:, :])
            nc.sync.dma_start(out=scand[0:ROWS, K : 2 * K], in_=s2[ROWS:P, :])
            nc.sync.dma_start(out=scand[ROWS:P, K : 2 * K], in_=s2[0:ROWS, :])

            # row top-64 from the 128 candidates
            nc.vector.max(out=s[:, 0:8], in_=scand[:, :])
            nc.vector.match_replace(
                out=w2[:, :], in_to_replace=s[:, 0:8], in_values=scand[:, :],
                imm_value=NEG,
            )
            for r in range(1, K // 8):
                sl = s[:, r * 8 : (r + 1) * 8]
                nc.vector.max(out=sl, in_=w2[:, :])
                nc.vector.match_replace(
                    out=w2[:, :], in_to_replace=sl, in_values=w2[:, :], imm_value=NEG
                )

            # M = s[:,0];  exp biases
            nc.scalar.mul(negm[:, :], s[:, 0:1], -1.0)
            nc.scalar.activation(
                negmtk[:, :], s[:, 0:1], Copy, bias=-KB_C, scale=-INV_TEMP
            )

            # E = exp(s - M), Z64, exclusive cumsum -> top-p cutoff sigma
            nc.scalar.activation(
                e64[:, :], s[:, :], Exp, bias=negm[:, :], scale=1.0,
                accum_out=z64[:, :],
            )
            nc.vector.tensor_tensor_scan(
                out=cum[:, :], data0=e64[:, :], data1=zeros64[:, :],
                initial=0.0, op0=A.add, op1=A.add,
            )
            nc.vector.tensor_sub(gx[:, :], cum[:, :], e64[:, :])
            nc.vector.tensor_scalar(
                out=thr[:, :], in0=z64[:, :], scalar1=float(TOP_P), scalar2=None,
                op0=A.mult,
            )
            nc.vector.tensor_scalar(
                out=km[:, :], in0=gx[:, :], scalar1=thr[:, :], scalar2=None,
                op0=A.is_le,
            )
            nc.vector.tensor_scalar(
                out=t64[:, :], in0=km[:, :], scalar1=-BIG, scalar2=BIG,
                op0=A.mult, op1=A.add,
            )
            nc.vector.tensor_mul(sk[:, :], s[:, :], km[:, :])
            nc.vector.tensor_add(ms[:, :], sk[:, :], t64[:, :])
            nc.vector.tensor_reduce(sig[:, :], ms[:, :], X, A.min)

            # keep mask;  probs path: xm = x + KILLC*mask, exp underflow kills
            nc.vector.tensor_scalar(
                out=mask[:, :], in0=x[:, :], scalar1=sig[:, :], scalar2=None,
                op0=A.is_ge,
            )
            nc.vector.scalar_tensor_tensor(
                out=xm[:, :], in0=mask[:, :], scalar=KILLC, in1=x[:, :],
                op0=A.mult, op1=A.add,
            )
            nc.scalar.activation(
                y[:, :], xm[:, :], Exp, bias=negmtk[:, :], scale=INV_TEMP,
                accum_out=zf[:, :],
            )
            # Zf = sum over both halves: swap-merge across partition groups
            nc.sync.dma_start(out=zfo[0:ROWS, :], in_=zf[ROWS:P, :])
            nc.sync.dma_start(out=zfo[ROWS:P, :], in_=zf[0:ROWS, :])
            nc.vector.tensor_add(zfr[:, :], zf[:, :], zfo[:, :])
            nc.vector.reciprocal(rzf[:, :], zfr[:, :])
            nc.vector.tensor_scalar_mul(y[:, :], y[:, :], rzf[:, :])

            # tokens: yqm = yq + (mask-1)*KILLC, exact for kept entries
            nc.vector.tensor_scalar(
                out=kbias[:, :], in0=mask[:, :], scalar1=KILLC, scalar2=-KILLC,
                op0=A.mult, op1=A.add,
            )
            nc.vector.tensor_add(yqm[:, :], yq[:, :], kbias[:, :])
            nc.vector.max(out=tm8[:, :], in_=yqm[:, :])
            nc.vector.max_index(out=tidx[:, :], in_max=tm8[:, :], in_values=yqm[:, :])

            # winner across halves (B strictly greater -> B, ties -> A)
            nc.sync.dma_start(out=tvb[:, :], in_=tm8[ROWS:P, 0:1])
            nc.sync.dma_start(out=tib[:, :], in_=tidx[ROWS:P, 0:1])
            nc.vector.tensor_tensor(
                out=wbf[:, :], in0=tvb[:, :], in1=tm8[0:ROWS, 0:1], op=A.is_gt
            )
            nc.vector.tensor_copy(tfa[:, :], tidx[0:ROWS, 0:1])
            nc.vector.tensor_copy(tfb[:, :], tib[:, :])
            nc.vector.tensor_scalar_add(tfb[:, :], tfb[:, :], float(H))
            nc.vector.tensor_sub(td[:, :], tfb[:, :], tfa[:, :])
            nc.vector.tensor_mul(td[:, :], td[:, :], wbf[:, :])
            nc.vector.tensor_add(tokf[:, :], tfa[:, :], td[:, :])

            # Stores: only live prob columns; everything else stays zero
            nc.sync.dma_start(out=probs_e[:, EOS : EOS + 1], in_=y[0:ROWS, 0:1])
            nc.sync.dma_start(
                out=probs_e[:, AUDIO_START : AUDIO_START + H - 1],
                in_=y[0:ROWS, 1:H],
            )
            nc.sync.dma_start(
                out=probs_e[:, AUDIO_START + H - 1 : V], in_=y[ROWS:P, :]
            )
            nc.sync.dma_start(out=tok_e[:, :], in_=tokf[:, :])

    _split_multi_waits(nc)
    return nc


def _build_nc_fast() -> bass.Bass:
    """Fast extraction variant: per-chunk top-8 pooling (values only) replaces
    the 8-round full-width extraction.  Exact when no chunk holds more than 8
    of its row's top-64 -- guaranteed by the host-side input check, which
    otherwise selects the safe builder.  Input DMA and cfg are striped so the
    chunk maxes start while later stripes are still loading; the keep-mask
    exp runs on GpSimd/ScalarE in parallel with the token argmax on DVE."""
    nc = bass.Bass()
    P = 2 * ROWS  # 128 partitions
    xin_e = nc.declare_dram_parameter("xin", [P * 2 * H], F32, isOutput=False)
    gs_e = nc.declare_dram_parameter("gs", [P * H], F32, isOutput=False)
    probs_e = nc.declare_dram_parameter("probs", [ROWS, V], F32, isOutput=True)
    tok_e = nc.declare_dram_parameter("tokens", [ROWS, 1], F32, isOutput=True)

    K = TOP_K
    NCH = len(CHUNKS)             # 21 chunks of the half row
    PW = 8 * NCH                  # 168: pooled candidates per half
    A = mybir.AluOpType
    X = mybir.AxisListType.X
    Exp = mybir.ActivationFunctionType.Exp
    Copy = mybir.ActivationFunctionType.Copy

    with TileContext(nc) as tc:
        with tc.tile_pool(name="pool", bufs=1) as pool:
            xin = pool.tile([P, 2 * H], F32)
            gs = pool.tile([P, H], F32)
            x = pool.tile([P, H], F32)
            xm = pool.tile([P, H], F32)
            y = pool.tile([P, H], F32)
            yq = pool.tile([P, H], F32)
            kbias = pool.tile([P, H], F32)
            yqm = pool.tile([P, H], F32)

            scand = pool.tile([P, 2 * PW], F32)  # both halves' chunk top-8 pools
            w2 = pool.tile([P, 2 * PW], F32)
            s = pool.tile([P, K], F32)           # row top-64, sorted desc
            e64 = pool.tile([P, K], F32)
            cum = pool.tile([P, K], F32)
            gx = pool.tile([P, K], F32)
            km = pool.tile([P, K], F32)
            t64 = pool.tile([P, K], F32)
            sk = pool.tile([P, K], F32)
            ms = pool.tile([P, K], F32)
            zeros64 = pool.tile([P, K], F32)

            negm = pool.tile([P, 1], F32)
            negmtk = pool.tile([P, 1], F32)
            z64 = pool.tile([P, 1], F32)
            thr = pool.tile([P, 1], F32)
            sig = pool.tile([P, 1], F32)
            zf = pool.tile([P, 1], F32)
            zfo = pool.tile([P, 1], F32)
            zfr = pool.tile([P, 1], F32)
            lnz = pool.tile([P, 1], F32)
            nlnz = pool.tile([P, 1], F32)
            bias2 = pool.tile([P, 1], F32)
            tm8 = pool.tile([P, 8], F32)
            tidx = pool.tile([P, 8], U32)
            tvb = pool.tile([ROWS, 1], F32)
            tib = pool.tile([ROWS, 1], U32)
            tfa = pool.tile([ROWS, 1], F32)
            tfb = pool.tile([ROWS, 1], F32)
            wbf = pool.tile([ROWS, 1], F32)
            td = pool.tile([ROWS, 1], F32)
            tokf = pool.tile([ROWS, 1], F32)

            # stripe loads from host-packed contiguous blocks (cond+unc pairs
            # first-needed-first): fully-linear DRAM reads, fat descriptors
            off = 0
            for si, (a, b) in enumerate(STRIPES):
                w = b - a
                for col0 in (a, H + a):
                    blk = xin_e[off : off + P * w].rearrange("(p c) -> p c", p=P)
                    nc.sync.dma_start(out=xin[:, col0 : col0 + w], in_=blk)
                    off += P * w
            gw = H // 2
            for gi, col0 in enumerate((0, gw)):
                blk = gs_e[gi * P * gw : (gi + 1) * P * gw].rearrange(
                    "(p c) -> p c", p=P
                )
                nc.gpsimd.dma_start(out=gs[:, col0 : col0 + gw], in_=blk)

            nc.gpsimd.memset(zeros64[:, :], 0.0)

            # cfg + chunk top-8s, stripe by stripe
            for si, (a, b) in enumerate(STRIPES):
                nc.vector.scalar_tensor_tensor(
                    out=x[:, a:b], in0=xin[:, a:b], scalar=float(CFG_SCALE),
                    in1=xin[:, H + a : H + b], op0=A.mult, op1=A.subtract,
                )
                for ci in range(*STRIPE_CHUNKS[si]):
                    ca, cb = CHUNKS[ci]
                    nc.vector.max(
                        out=scand[:, 8 * ci : 8 * ci + 8], in_=x[:, ca:cb]
                    )

            # tokens numerator off the DVE critical path
            nc.gpsimd.tensor_add(yq[:, :], x[:, :], gs[:, :])

            # merge both halves' pools into both partition groups
            nc.sync.dma_start(
                out=scand[0:ROWS, PW : 2 * PW], in_=scand[ROWS:P, 0:PW]
            )
            nc.sync.dma_start(
                out=scand[ROWS:P, PW : 2 * PW], in_=scand[0:ROWS, 0:PW]
            )

            # row top-64 from the 512 pooled candidates
            nc.vector.max(out=s[:, 0:8], in_=scand[:, :])
            nc.vector.match_replace(
                out=w2[:, :], in_to_replace=s[:, 0:8], in_values=scand[:, :],
                imm_value=NEG,
            )
            for r in range(1, K // 8):
                sl = s[:, r * 8 : (r + 1) * 8]
                nc.vector.max(out=sl, in_=w2[:, :])
                nc.vector.match_replace(
                    out=w2[:, :], in_to_replace=sl, in_values=w2[:, :], imm_value=NEG
                )

            # M = s[:,0];  exp biases
            nc.scalar.mul(negm[:, :], s[:, 0:1], -1.0)
            nc.scalar.mul(negmtk[:, :], s[:, 0:1], -INV_TEMP)

            # E = exp(s - M), Z64, exclusive cumsum -> top-p cutoff sigma
            nc.scalar.activation(
                e64[:, :], s[:, :], Exp, bias=negm[:, :], scale=1.0,
                accum_out=z64[:, :],
            )
            nc.vector.tensor_tensor_scan(
                out=cum[:, :], data0=e64[:, :], data1=zeros64[:, :],
                initial=0.0, op0=A.add, op1=A.add,
            )
            nc.vector.tensor_sub(gx[:, :], cum[:, :], e64[:, :])
            nc.vector.tensor_scalar(
                out=thr[:, :], in0=z64[:, :], scalar1=float(TOP_P), scalar2=None,
                op0=A.mult,
            )
            nc.vector.tensor_scalar(
                out=km[:, :], in0=gx[:, :], scalar1=thr[:, :], scalar2=None,
                op0=A.is_le,
            )
            nc.vector.tensor_scalar(
                out=t64[:, :], in0=km[:, :], scalar1=-BIG, scalar2=BIG,
                op0=A.mult, op1=A.add,
            )
            nc.vector.tensor_mul(sk[:, :], s[:, :], km[:, :])
            nc.vector.tensor_add(ms[:, :], sk[:, :], t64[:, :])
            nc.vector.tensor_reduce(sig[:, :], ms[:, :], X, A.min)

            # fused keep mask -> exact additive kill bias (0 kept, -512 dropped)
            nc.vector.tensor_scalar(
                out=kbias[:, :], in0=x[:, :], scalar1=sig[:, :], scalar2=-KILLC,
                op0=A.is_lt, op1=A.mult,
            )
            nc.vector.tensor_add(xm[:, :], x[:, :], kbias[:, :])
            # probs = exp((xm - M)/temp)/Zf; dropped entries underflow to 0 and
            # the ACT accumulator therefore yields the masked Zf directly
            nc.scalar.activation(
                y[:, :], xm[:, :], Exp, bias=negmtk[:, :], scale=INV_TEMP,
                accum_out=zf[:, :],
            )
            nc.sync.dma_start(out=zfo[0:ROWS, :], in_=zf[ROWS:P, :])
            nc.sync.dma_start(out=zfo[ROWS:P, :], in_=zf[0:ROWS, :])
            # normalization entirely on ScalarE: P = exp((xm-M)/T - ln Zf);
            # DVE runs the token argmax in parallel, stores chase only ACT
            nc.scalar.add(zfr[:, :], zf[:, :], zfo[:, :])
            nc.scalar.activation(
                lnz[:, :], zfr[:, :], mybir.ActivationFunctionType.Ln
            )
            nc.scalar.mul(nlnz[:, :], lnz[:, :], -1.0)
            nc.scalar.add(bias2[:, :], negmtk[:, :], nlnz[:, :])
            nc.scalar.activation(
                y[:, :], xm[:, :], Exp, bias=bias2[:, :], scale=INV_TEMP
            )
            nc.sync.dma_start(out=probs_e[:, EOS : EOS + 1], in_=y[0:ROWS, 0:1])
            RH = ROWS // 2
            for q in range(2):
                r0, r1 = q * RH, (q + 1) * RH
                nc.sync.dma_start(
                    out=probs_e[r0:r1, AUDIO_START : AUDIO_START + H - 1],
                    in_=y[r0:r1, 1:H],
                )
                nc.sync.dma_start(
                    out=probs_e[r0:r1, AUDIO_START + H - 1 : V],
                    in_=y[ROWS + r0 : ROWS + r1, :],
                )

            nc.vector.tensor_add(yqm[:, :], yq[:, :], kbias[:, :])
            nc.vector.max(out=tm8[:, :], in_=yqm[:, :])
            nc.vector.max_index(out=tidx[:, :], in_max=tm8[:, :], in_values=yqm[:, :])

            # winner across halves (B strictly greater -> B, ties -> A)
            nc.sync.dma_start(out=tvb[:, :], in_=tm8[ROWS:P, 0:1])
            nc.sync.dma_start(out=tib[:, :], in_=tidx[ROWS:P, 0:1])
            nc.vector.tensor_tensor(
                out=wbf[:, :], in0=tvb[:, :], in1=tm8[0:ROWS, 0:1], op=A.is_gt
            )
            nc.vector.tensor_copy(tfa[:, :], tidx[0:ROWS, 0:1])
            nc.vector.tensor_copy(tfb[:, :], tib[:, :])
            nc.vector.tensor_scalar_add(tfb[:, :], tfb[:, :], float(H))
            nc.vector.tensor_sub(td[:, :], tfb[:, :], tfa[:, :])
            nc.vector.tensor_mul(td[:, :], td[:, :], wbf[:, :])
            nc.vector.tensor_add(tokf[:, :], tfa[:, :], td[:, :])

            nc.sync.dma_start(out=tok_e[:, :], in_=tokf[:, :])

    _split_multi_waits(nc)
    return nc


def fast_path_ok(live: np.ndarray) -> bool:
    """True iff per-chunk top-8 pooling recovers every row's exact top-64,
    i.e. no 64-wide chunk of either half holds more than 8 values >= the
    row's 64th-largest (ties counted conservatively)."""
    x = (live[:, 0] * np.float32(2.0)) - live[:, 1]
    tau = np.partition(x, L - TOP_K, axis=1)[:, L - TOP_K]
    ge = x >= tau[:, None]
    if not np.all(ge.sum(axis=1) == TOP_K):
        return False  # ties at the boundary: let the safe path handle them
    for half in range(2):
        g = ge[:, half * H : (half + 1) * H]
        for a, b in CHUNKS:
            if g[:, a:b].sum(axis=1).max() > 8:
                return False
    return True


def _get_nc(fast: bool) -> bass.Bass:
    key = "nc_fast" if fast else "nc_safe"
    if key not in _CACHE:
        _CACHE[key] = _build_nc_fast() if fast else _build_nc_safe()
    return _CACHE[key]


def _gumbel_live_scaled() -> np.ndarray:
    """temp * gumbel noise of jax.random.categorical at the live columns,
    bit-exact vs the reference (jax threefry on CPU), [T, L] float32."""
    if "gs" in _CACHE:
        return _CACHE["gs"]
    import jax
    import jax.numpy as jnp

    with jax.default_device(jax.devices("cpu")[0]):
        keys = jax.random.split(jax.random.key(1), T)
        gum = jax.jit(
            jax.vmap(lambda k: jax.random.gumbel(k, (V,), jnp.float32))
        )(keys)
        gum = np.asarray(gum)
    gl = np.empty((T, L), np.float32)
    gl[:, 0] = gum[:, EOS]
    gl[:, 1:] = gum[:, AUDIO_START:]
    gl *= np.float32(TEMPERATURE)
    _CACHE["gs"] = gl
    return gl


def make_live(logits: np.ndarray) -> np.ndarray:
    logits = np.asarray(logits, dtype=np.float32)
    live = np.empty((T, 2, L), np.float32)
    live[:, :, 0] = logits[:, :, EOS]
    live[:, :, 1:] = logits[:, :, AUDIO_START:]
    # EOS kill for steps <= MIN_TOKENS: force cond so 2*cond - uncond ~ -1e30,
    # far below any live logit -> never in the top-64, prob exactly 0.
    live[: MIN_TOKENS + 1, 0, 0] = EOS_KILL / 2
    return live


def make_in_maps(live: np.ndarray, fast: bool) -> list[dict[str, np.ndarray]]:
    gs = _gumbel_live_scaled()
    maps = []
    for c in range(N_CORES):
        r0, r1 = c * ROWS, (c + 1) * ROWS
        # partition p<64: row p's live cols [0:H); p>=64: row p-64's [H:L)
        xin2 = np.empty((2 * ROWS, 2 * H), np.float32)
        xin2[:ROWS, 0:H] = live[r0:r1, 0, 0:H]
        xin2[:ROWS, H:] = live[r0:r1, 1, 0:H]
        xin2[ROWS:, 0:H] = live[r0:r1, 0, H:L]
        xin2[ROWS:, H:] = live[r0:r1, 1, H:L]
        gs2 = np.empty((2 * ROWS, H), np.float32)
        gs2[:ROWS] = gs[r0:r1, 0:H]
        gs2[ROWS:] = gs[r0:r1, H:L]
        if fast:
            # stripe-contiguous packing mirroring the fast builder's reads
            blocks = []
            for a, b in STRIPES:
                blocks.append(xin2[:, a:b].ravel())
                blocks.append(xin2[:, H + a : H + b].ravel())
            xinp = np.concatenate(blocks)
            gw = H // 2
            gsp = np.concatenate(
                [gs2[:, 0:gw].ravel(), gs2[:, gw:H].ravel()]
            )
            maps.append({"xin": xinp, "gs": gsp})
        else:
            maps.append({"xin": xin2, "gs": gs2})
    return maps


def postprocess(results: list[dict[str, np.ndarray]]):
    probs = np.concatenate([r["probs"] for r in results], axis=0)
    tidx = np.concatenate(
        [r["tokens"][:, 0].astype(np.int64) for r in results], axis=0
    )
    tokens = np.where(tidx == 0, EOS, AUDIO_START - 1 + tidx).astype(np.int32)
    return tokens, probs


def kernel(logits: np.ndarray):
    live = make_live(logits)
    fast = os.environ.get("SAMPLER_FORCE_PATH", "")
    use_fast = fast_path_ok(live) if fast == "" else (fast == "fast")
    nc = _get_nc(use_fast)
    in_maps = make_in_maps(live, use_fast)
    res = bass_utils.run_bass_kernel_spmd(
        nc, in_maps, core_ids=list(range(N_CORES))
    )
    _CACHE["last_run"] = res
    return postprocess(res.results)


# revision 42
# speedup vs baseline: 1.5706x; 1.0366x over previous
"""Trainium2 Bass kernel for nn_ACE15TEModel_41824391528638 (CFG sampling).

Reference semantics per decode step t over vocab V=155776:
  cfg = uncond + 2.0*(cond - uncond)
  mask tokens < AUDIO_START to -inf (EOS restored when t > MIN_TOKENS)
  top-k(64) filter, top-p(0.9) nucleus filter, temperature 0.85
  probs = softmax(filtered); token = categorical(key_t, filtered) [Gumbel argmax]

Structure exploited:
  * Only the L = V-AUDIO_START+1 = 4108 "live" columns (EOS + audio tokens)
    can survive the audio mask; every other prob column is exactly 0 and no
    dead input column is ever read.  The runner pre-zeros ExternalOutput
    buffers (native path zero-fills, PJRT path donates zero buffers), so the
    kernel writes only the live columns of the full [T, V] probs output.
  * The final keep set is {v : v >= sigma}, sigma derived from the exact
    sorted top-64 values per row via the exp-cumsum crossing of top-p.
  * The Gumbel noise of jax.random.categorical is input-independent ->
    precomputed bit-exactly on host (jax CPU threefry) as a kernel input.

Sharding: steps (T=512) data-parallel over 8 cores; per core 64 steps, each
row split into two 2054-wide halves on SBUF partitions r and r+64.

Fast path (used whenever the host check passes, incl. the spec's input):
  striped DMA + cfg -> per-96-col-chunk top-8 pools (DVE max8) -> cross-
  partition pool merge -> exact row top-64 via 8 rounds of max8+match_replace
  on the 336-wide pool -> top-p cutoff sigma -> additive -512 kill bias ->
  probs = exp((x-M)/T - lnZf) on ScalarE (stores chase only ACT) while DVE
  runs the Gumbel argmax (max8 + max_index) -> merged winner across halves.
  Exactness condition (no chunk holds >8 of its row's top-64) is verified on
  host against the actual input; otherwise the safe builder (full-width
  8-round extraction per half) is selected, which is exact unconditionally.
"""

import os
import sys

import numpy as np

for _p in (
    "/root/.axon_site",
    "/root/.axon_site/_ro/trn_rl_repo",
    "/root/.axon_site/_ro/pypackages",
    "/opt/trn_rl_repo",
):
    if os.path.isdir(_p) and _p not in sys.path:
        sys.path.append(_p)

import concourse.bass as bass
import concourse.mybir as mybir
from concourse import bass_utils
from concourse.tile import TileContext

# Problem constants (hardcoded per spec)
T = 512
V = 155776
AUDIO_START = 151669
EOS = 151645
MIN_TOKENS = 1
CFG_SCALE = 2.0
TEMPERATURE = 0.85
TOP_P = 0.9
TOP_K = 64

N_CORES = 8
ROWS = T // N_CORES            # 64 steps per core
L = V - AUDIO_START + 1        # 4108 live columns: [EOS, AUDIO_START..V)
H = L // 2                     # 2054: row half width (two partitions per row)

NEG = -3.4028235e38            # float32 min (extraction sentinel)
EOS_KILL = -1.0e30             # finite stand-in for -inf EOS mask (avoids inf/nan)
BIG = 3.0e38
KILLC = 512.0                  # additive keep-mask bias: power of two, exact for
                               # {0,1} masks; exp((x-512-M)/temp) underflows to 0
INV_TEMP = 1.0 / TEMPERATURE
KB_C = KILLC * INV_TEMP        # folded into the exp bias so kept entries cancel
# 96-wide chunks over each half row (last takes the 134-col remainder).  The
# fast path pools each chunk's top-8; exact iff no chunk holds >8 of the
# row's top-64 (host-checked per input, safe builder otherwise).
CHUNKS = [(c * 96, (c + 1) * 96) for c in range(20)] + [(1920, H)]
# DMA/compute stripes (in chunks): fat stripes keep DMA descriptors >=2.6KB
# (descriptor rate, not bandwidth, limits thin transfers)
STRIPE_CHUNKS = [(0, 7), (7, 14), (14, 21)]
STRIPES = [(CHUNKS[lo][0], CHUNKS[hi - 1][1]) for lo, hi in STRIPE_CHUNKS]

F32 = mybir.dt.float32
U32 = mybir.dt.uint32

_CACHE: dict = {}


def _ensure_trace_hooks() -> None:
    """If the environment enables BASS_TRACE, run_bass_kernel_spmd imports
    antenv.axon_hooks, which some images lack; synthesize a compatible module
    (wired to the axon NTFF ctypes hook when available) so tracing works
    instead of crashing.  No-op when tracing is off or the module exists."""
    try:
        import antenv.axon_hooks  # noqa: F401
        return
    except ImportError:
        pass
    import types

    try:
        import antenv
    except ImportError:
        return
    mod = types.ModuleType("antenv.axon_hooks")
    holder: dict = {"hook": None, "tried": False}

    def set_axon_ntff_profile_hook(h):
        holder["hook"] = h

    def get_axon_ntff_profile_hook():
        if holder["hook"] is None and not holder["tried"]:
            holder["tried"] = True
            try:
                from trn_agent_boot.trn_boot import _ntff_profile_via_ctypes

                so = "/opt/axon/libaxon_pjrt.so"
                if os.path.exists(so):
                    holder["hook"] = _ntff_profile_via_ctypes(so)
            except Exception:
                pass
        return holder["hook"]

    mod.set_axon_ntff_profile_hook = set_axon_ntff_profile_hook
    mod.get_axon_ntff_profile_hook = get_axon_ntff_profile_hook
    sys.modules["antenv.axon_hooks"] = mod
    antenv.axon_hooks = mod


_ensure_trace_hooks()

# Artifact upload has no remote store in sandboxed runs; degrade locally.
_orig_upload = bass_utils.upload_artifacts


def _safe_upload(tmpdir: str) -> str:
    try:
        return _orig_upload(tmpdir)
    except Exception:
        return tmpdir


bass_utils.upload_artifacts = _safe_upload


def _split_multi_waits(nc: bass.Bass) -> None:
    """This walrus build allows one semaphore wait per compute instruction.
    Hoist extra on_wait entries into standalone InstEventSemaphore ops placed
    just before the instruction on the same engine (engines are in-order, so
    semantics are identical)."""
    skip = (mybir.InstEventSemaphore,)
    for fn in nc.m.functions:
        for blk in fn.blocks:
            new_insts = []
            for inst in blk.instructions:
                si = getattr(inst, "sync_info", None)
                if (
                    si is not None
                    and si.on_wait is not None
                    and len(si.on_wait) > 1
                    and not isinstance(inst, skip)
                ):
                    for w in si.on_wait[:-1]:
                        ev = mybir.InstEventSemaphore(
                            name=f"{inst.name}-wsplit-{w.id}",
                            ins=[],
                            outs=[],
                            sync_info=mybir.SyncInfo(on_wait=[w], on_update=[]),
                        )
                        ev.engine = inst.engine
                        new_insts.append(ev)
                    inst.sync_info = mybir.SyncInfo(
                        on_wait=[si.on_wait[-1]], on_update=si.on_update
                    )
                new_insts.append(inst)
            blk.instructions = new_insts


def _build_nc_safe() -> bass.Bass:
    """128-partition layout: step row r occupies partitions r (live cols
    [0:H), i.e. EOS + audio[0:H-1)) and r+64 (live cols [H:L)).  Per-half
    exact top-64 extraction, then the two halves' candidates are merged into
    both partition groups (redundant small-op compute beats cross-partition
    broadcast latency)."""
    nc = bass.Bass()
    xin_e = nc.declare_dram_parameter("xin", [2 * ROWS, 2 * H], F32, isOutput=False)
    gs_e = nc.declare_dram_parameter("gs", [2 * ROWS, H], F32, isOutput=False)
    probs_e = nc.declare_dram_parameter("probs", [ROWS, V], F32, isOutput=True)
    tok_e = nc.declare_dram_parameter("tokens", [ROWS, 1], F32, isOutput=True)

    K = TOP_K
    P = 2 * ROWS  # 128 partitions
    A = mybir.AluOpType
    X = mybir.AxisListType.X
    Exp = mybir.ActivationFunctionType.Exp
    Copy = mybir.ActivationFunctionType.Copy

    with TileContext(nc) as tc:
        with tc.tile_pool(name="pool", bufs=1) as pool:
            xin = pool.tile([P, 2 * H], F32)
            gs = pool.tile([P, H], F32)
            x = pool.tile([P, H], F32)
            w = pool.tile([P, H], F32)
            mask = pool.tile([P, H], F32)
            xm = pool.tile([P, H], F32)
            y = pool.tile([P, H], F32)
            yq = pool.tile([P, H], F32)
            kbias = pool.tile([P, H], F32)
            yqm = pool.tile([P, H], F32)

            s2 = pool.tile([P, K], F32)       # per-half top-64, sorted desc
            scand = pool.tile([P, 2 * K], F32)  # both halves' candidates
            w2 = pool.tile([P, 2 * K], F32)
            s = pool.tile([P, K], F32)        # row top-64 (same in both groups)
            e64 = pool.tile([P, K], F32)
            cum = pool.tile([P, K], F32)
            gx = pool.tile([P, K], F32)
            km = pool.tile([P, K], F32)
            t64 = pool.tile([P, K], F32)
            sk = pool.tile([P, K], F32)
            ms = pool.tile([P, K], F32)
            zeros64 = pool.tile([P, K], F32)

            negm = pool.tile([P, 1], F32)
            negmtk = pool.tile([P, 1], F32)
            z64 = pool.tile([P, 1], F32)
            thr = pool.tile([P, 1], F32)
            sig = pool.tile([P, 1], F32)
            zf = pool.tile([P, 1], F32)
            zfo = pool.tile([P, 1], F32)
            zfr = pool.tile([P, 1], F32)
            rzf = pool.tile([P, 1], F32)
            tm8 = pool.tile([P, 8], F32)
            tidx = pool.tile([P, 8], U32)
            tvb = pool.tile([ROWS, 1], F32)
            tib = pool.tile([ROWS, 1], U32)
            tfa = pool.tile([ROWS, 1], F32)
            tfb = pool.tile([ROWS, 1], F32)
            wbf = pool.tile([ROWS, 1], F32)
            td = pool.tile([ROWS, 1], F32)
            tokf = pool.tile([ROWS, 1], F32)

            HH = H // 2
            # Loads: cond/unc quarters on separate queues so x can start early
            nc.sync.dma_start(out=xin[:, 0:HH], in_=xin_e[:, 0:HH])
            nc.sync.dma_start(out=xin[:, H : H + HH], in_=xin_e[:, H : H + HH])
            nc.sync.dma_start(out=xin[:, HH:H], in_=xin_e[:, HH:H])
            nc.sync.dma_start(out=xin[:, H + HH : 2 * H], in_=xin_e[:, H + HH : 2 * H])
            nc.sync.dma_start(out=gs[:, 0:HH], in_=gs_e[:, 0:HH])
            nc.sync.dma_start(out=gs[:, HH:H], in_=gs_e[:, HH:H])

            nc.gpsimd.memset(zeros64[:, :], 0.0)

            # x = 2*cond - uncond, in two column stripes
            for c0, c1 in ((0, HH), (HH, H)):
                nc.vector.scalar_tensor_tensor(
                    out=x[:, c0:c1], in0=xin[:, c0:c1], scalar=float(CFG_SCALE),
                    in1=xin[:, H + c0 : H + c1], op0=A.mult, op1=A.subtract,
                )

            # tokens numerator (off the DVE critical path)
            nc.gpsimd.tensor_add(yq[:, :], x[:, :], gs[:, :])

            # per-half exact top-64: 8 rounds of max8 + match_replace
            nc.vector.max(out=s2[:, 0:8], in_=x[:, :])
            nc.vector.match_replace(
                out=w[:, :], in_to_replace=s2[:, 0:8], in_values=x[:, :],
                imm_value=NEG,
            )
            for r in range(1, K // 8):
                sl = s2[:, r * 8 : (r + 1) * 8]
                nc.vector.max(out=sl, in_=w[:, :])
                nc.vector.match_replace(
                    out=w[:, :], in_to_replace=sl, in_values=w[:, :], imm_value=NEG
                )

            # merge the two halves' top-64 into both partition groups
            nc.vector.tensor_copy(scand[:, 0:K], s2[:, :])
            nc.sync.dma_start(out=scand[0:ROWS, K : 2 * K], in_=s2[ROWS:P, :])
            nc.sync.dma_start(out=scand[ROWS:P, K : 2 * K], in_=s2[0:ROWS, :])

            # row top-64 from the 128 candidates
            nc.vector.max(out=s[:, 0:8], in_=scand[:, :])
            nc.vector.match_replace(
                out=w2[:, :], in_to_replace=s[:, 0:8], in_values=scand[:, :],
                imm_value=NEG,
            )
            for r in range(1, K // 8):
                sl = s[:, r * 8 : (r + 1) * 8]
                nc.vector.max(out=sl, in_=w2[:, :])
                nc.vector.match_replace(
                    out=w2[:, :], in_to_replace=sl, in_values=w2[:, :], imm_value=NEG
                )

            # M = s[:,0];  exp biases
            nc.scalar.mul(negm[:, :], s[:, 0:1], -1.0)
            nc.scalar.activation(
                negmtk[:, :], s[:, 0:1], Copy, bias=-KB_C, scale=-INV_TEMP
            )

            # E = exp(s - M), Z64, exclusive cumsum -> top-p cutoff sigma
            nc.scalar.activation(
                e64[:, :], s[:, :], Exp, bias=negm[:, :], scale=1.0,
                accum_out=z64[:, :],
            )
            nc.vector.tensor_tensor_scan(
                out=cum[:, :], data0=e64[:, :], data1=zeros64[:, :],
                initial=0.0, op0=A.add, op1=A.add,
            )
            nc.vector.tensor_sub(gx[:, :], cum[:, :], e64[:, :])
            nc.vector.tensor_scalar(
                out=thr[:, :], in0=z64[:, :], scalar1=float(TOP_P), scalar2=None,
                op0=A.mult,
            )
            nc.vector.tensor_scalar(
                out=km[:, :], in0=gx[:, :], scalar1=thr[:, :], scalar2=None,
                op0=A.is_le,
            )
            nc.vector.tensor_scalar(
                out=t64[:, :], in0=km[:, :], scalar1=-BIG, scalar2=BIG,
                op0=A.mult, op1=A.add,
            )
            nc.vector.tensor_mul(sk[:, :], s[:, :], km[:, :])
            nc.vector.tensor_add(ms[:, :], sk[:, :], t64[:, :])
            nc.vector.tensor_reduce(sig[:, :], ms[:, :], X, A.min)

            # keep mask;  probs path: xm = x + KILLC*mask, exp underflow kills
            nc.vector.tensor_scalar(
                out=mask[:, :], in0=x[:, :], scalar1=sig[:, :], scalar2=None,
                op0=A.is_ge,
            )
            nc.vector.scalar_tensor_tensor(
                out=xm[:, :], in0=mask[:, :], scalar=KILLC, in1=x[:, :],
                op0=A.mult, op1=A.add,
            )
            nc.scalar.activation(
                y[:, :], xm[:, :], Exp, bias=negmtk[:, :], scale=INV_TEMP,
                accum_out=zf[:, :],
            )
            # Zf = sum over both halves: swap-merge across partition groups
            nc.sync.dma_start(out=zfo[0:ROWS, :], in_=zf[ROWS:P, :])
            nc.sync.dma_start(out=zfo[ROWS:P, :], in_=zf[0:ROWS, :])
            nc.vector.tensor_add(zfr[:, :], zf[:, :], zfo[:, :])
            nc.vector.reciprocal(rzf[:, :], zfr[:, :])
            nc.vector.tensor_scalar_mul(y[:, :], y[:, :], rzf[:, :])

            # tokens: yqm = yq + (mask-1)*KILLC, exact for kept entries
            nc.vector.tensor_scalar(
                out=kbias[:, :], in0=mask[:, :], scalar1=KILLC, scalar2=-KILLC,
                op0=A.mult, op1=A.add,
            )
            nc.vector.tensor_add(yqm[:, :], yq[:, :], kbias[:, :])
            nc.vector.max(out=tm8[:, :], in_=yqm[:, :])
            nc.vector.max_index(out=tidx[:, :], in_max=tm8[:, :], in_values=yqm[:, :])

            # winner across halves (B strictly greater -> B, ties -> A)
            nc.sync.dma_start(out=tvb[:, :], in_=tm8[ROWS:P, 0:1])
            nc.sync.dma_start(out=tib[:, :], in_=tidx[ROWS:P, 0:1])
            nc.vector.tensor_tensor(
                out=wbf[:, :], in0=tvb[:, :], in1=tm8[0:ROWS, 0:1], op=A.is_gt
            )
            nc.vector.tensor_copy(tfa[:, :], tidx[0:ROWS, 0:1])
            nc.vector.tensor_copy(tfb[:, :], tib[:, :])
            nc.vector.tensor_scalar_add(tfb[:, :], tfb[:, :], float(H))
            nc.vector.tensor_sub(td[:, :], tfb[:, :], tfa[:, :])
            nc.vector.tensor_mul(td[:, :], td[:, :], wbf[:, :])
            nc.vector.tensor_add(tokf[:, :], tfa[:, :], td[:, :])

            # Stores: only live prob columns; everything else stays zero
            nc.sync.dma_start(out=probs_e[:, EOS : EOS + 1], in_=y[0:ROWS, 0:1])
            nc.sync.dma_start(
                out=probs_e[:, AUDIO_START : AUDIO_START + H - 1],
                in_=y[0:ROWS, 1:H],
            )
            nc.sync.dma_start(
                out=probs_e[:, AUDIO_START + H - 1 : V], in_=y[ROWS:P, :]
            )
            nc.sync.dma_start(out=tok_e[:, :], in_=tokf[:, :])

    _split_multi_waits(nc)
    return nc


def _build_nc_fast() -> bass.Bass:
    """Fast extraction variant: per-chunk top-8 pooling (values only) replaces
    the 8-round full-width extraction.  Exact when no chunk holds more than 8
    of its row's top-64 -- guaranteed by the host-side input check, which
    otherwise selects the safe builder.  Input DMA and cfg are striped so the
    chunk maxes start while later stripes are still loading; the keep-mask
    exp runs on GpSimd/ScalarE in parallel with the token argmax on DVE."""
    nc = bass.Bass()
    P = 2 * ROWS  # 128 partitions
    xin_e = nc.declare_dram_parameter("xin", [P * 2 * H], F32, isOutput=False)
    gs_e = nc.declare_dram_parameter("gs", [P * H], F32, isOutput=False)
    probs_e = nc.declare_dram_parameter("probs", [ROWS, V], F32, isOutput=True)
    tok_e = nc.declare_dram_parameter("tokens", [ROWS, 1], F32, isOutput=True)

    K = TOP_K
    NCH = len(CHUNKS)             # 21 chunks of the half row
    PW = 8 * NCH                  # 168: pooled candidates per half
    A = mybir.AluOpType
    X = mybir.AxisListType.X
    Exp = mybir.ActivationFunctionType.Exp
    Copy = mybir.ActivationFunctionType.Copy

    with TileContext(nc) as tc:
        with tc.tile_pool(name="pool", bufs=1) as pool:
            xin = pool.tile([P, 2 * H], F32)
            gs = pool.tile([P, H], F32)
            x = pool.tile([P, H], F32)
            xm = pool.tile([P, H], F32)
            y = pool.tile([P, H], F32)
            yq = pool.tile([P, H], F32)
            kbias = pool.tile([P, H], F32)
            yqm = pool.tile([P, H], F32)

            scand = pool.tile([P, 2 * PW], F32)  # both halves' chunk top-8 pools
            w2 = pool.tile([P, 2 * PW], F32)
            s = pool.tile([P, K], F32)           # row top-64, sorted desc
            e64 = pool.tile([P, K], F32)
            cum = pool.tile([P, K], F32)
            gx = pool.tile([P, K], F32)
            km = pool.tile([P, K], F32)
            t64 = pool.tile([P, K], F32)
            sk = pool.tile([P, K], F32)
            ms = pool.tile([P, K], F32)
            zeros64 = pool.tile([P, K], F32)

            negm = pool.tile([P, 1], F32)
            negmtk = pool.tile([P, 1], F32)
            z64 = pool.tile([P, 1], F32)
            thr = pool.tile([P, 1], F32)
            sig = pool.tile([P, 1], F32)
            kb64 = pool.tile([P, K], F32)
            sm2 = pool.tile([P, K], F32)
            et = pool.tile([P, K], F32)
            zf = pool.tile([P, 1], F32)
            lnz = pool.tile([P, 1], F32)
            nlnz = pool.tile([P, 1], F32)
            bias2 = pool.tile([P, 1], F32)
            tm8 = pool.tile([P, 8], F32)
            tidx = pool.tile([P, 8], U32)
            tvb = pool.tile([ROWS, 1], F32)
            tib = pool.tile([ROWS, 1], U32)
            tfa = pool.tile([ROWS, 1], F32)
            tfb = pool.tile([ROWS, 1], F32)
            wbf = pool.tile([ROWS, 1], F32)
            td = pool.tile([ROWS, 1], F32)
            tokf = pool.tile([ROWS, 1], F32)

            # stripe loads from host-packed contiguous blocks (cond+unc pairs
            # first-needed-first): fully-linear DRAM reads, fat descriptors
            off = 0
            for si, (a, b) in enumerate(STRIPES):
                w = b - a
                for col0 in (a, H + a):
                    blk = xin_e[off : off + P * w].rearrange("(p c) -> p c", p=P)
                    nc.sync.dma_start(out=xin[:, col0 : col0 + w], in_=blk)
                    off += P * w
            gw = H // 2
            for gi, col0 in enumerate((0, gw)):
                blk = gs_e[gi * P * gw : (gi + 1) * P * gw].rearrange(
                    "(p c) -> p c", p=P
                )
                nc.gpsimd.dma_start(out=gs[:, col0 : col0 + gw], in_=blk)

            nc.gpsimd.memset(zeros64[:, :], 0.0)

            # cfg + chunk top-8s, stripe by stripe
            for si, (a, b) in enumerate(STRIPES):
                nc.vector.scalar_tensor_tensor(
                    out=x[:, a:b], in0=xin[:, a:b], scalar=float(CFG_SCALE),
                    in1=xin[:, H + a : H + b], op0=A.mult, op1=A.subtract,
                )
                for ci in range(*STRIPE_CHUNKS[si]):
                    ca, cb = CHUNKS[ci]
                    nc.vector.max(
                        out=scand[:, 8 * ci : 8 * ci + 8], in_=x[:, ca:cb]
                    )

            # tokens numerator off the DVE critical path
            nc.gpsimd.tensor_add(yq[:, :], x[:, :], gs[:, :])

            # merge both halves' pools into both partition groups
            nc.sync.dma_start(
                out=scand[0:ROWS, PW : 2 * PW], in_=scand[ROWS:P, 0:PW]
            )
            nc.sync.dma_start(
                out=scand[ROWS:P, PW : 2 * PW], in_=scand[0:ROWS, 0:PW]
            )

            # row top-64 from the 512 pooled candidates
            nc.vector.max(out=s[:, 0:8], in_=scand[:, :])
            nc.vector.match_replace(
                out=w2[:, :], in_to_replace=s[:, 0:8], in_values=scand[:, :],
                imm_value=NEG,
            )
            for r in range(1, K // 8):
                sl = s[:, r * 8 : (r + 1) * 8]
                nc.vector.max(out=sl, in_=w2[:, :])
                nc.vector.match_replace(
                    out=w2[:, :], in_to_replace=sl, in_values=w2[:, :], imm_value=NEG
                )

            # M = s[:,0];  exp biases
            nc.scalar.mul(negm[:, :], s[:, 0:1], -1.0)
            nc.scalar.mul(negmtk[:, :], s[:, 0:1], -INV_TEMP)

            # E = exp(s - M), Z64, exclusive cumsum -> top-p cutoff sigma
            nc.scalar.activation(
                e64[:, :], s[:, :], Exp, bias=negm[:, :], scale=1.0,
                accum_out=z64[:, :],
            )
            nc.vector.tensor_tensor_scan(
                out=cum[:, :], data0=e64[:, :], data1=zeros64[:, :],
                initial=0.0, op0=A.add, op1=A.add,
            )
            nc.vector.tensor_sub(gx[:, :], cum[:, :], e64[:, :])
            nc.vector.tensor_scalar(
                out=thr[:, :], in0=z64[:, :], scalar1=float(TOP_P), scalar2=None,
                op0=A.mult,
            )
            nc.vector.tensor_scalar(
                out=km[:, :], in0=gx[:, :], scalar1=thr[:, :], scalar2=None,
                op0=A.is_le,
            )
            nc.vector.tensor_scalar(
                out=t64[:, :], in0=km[:, :], scalar1=-BIG, scalar2=BIG,
                op0=A.mult, op1=A.add,
            )
            nc.vector.tensor_mul(sk[:, :], s[:, :], km[:, :])
            nc.vector.tensor_add(ms[:, :], sk[:, :], t64[:, :])
            nc.vector.tensor_reduce(sig[:, :], ms[:, :], X, A.min)

            # Zf from the sorted top-64 alone: km is the keep mask on s and s
            # is replicated in both partition groups, so no full-width exp and
            # no cross-partition merge are needed at all.
            nc.vector.tensor_scalar(
                out=kb64[:, :], in0=km[:, :], scalar1=KILLC, scalar2=-KILLC,
                op0=A.mult, op1=A.add,
            )
            nc.vector.tensor_add(sm2[:, :], s[:, :], kb64[:, :])
            nc.scalar.activation(
                et[:, :], sm2[:, :], Exp, bias=negmtk[:, :], scale=INV_TEMP,
                accum_out=zf[:, :],
            )
            nc.scalar.activation(
                lnz[:, :], zf[:, :], mybir.ActivationFunctionType.Ln
            )
            nc.scalar.mul(nlnz[:, :], lnz[:, :], -1.0)
            nc.scalar.add(bias2[:, :], negmtk[:, :], nlnz[:, :])

            # fused keep mask -> exact additive kill bias (0 kept, -512 dropped)
            nc.vector.tensor_scalar(
                out=kbias[:, :], in0=x[:, :], scalar1=sig[:, :], scalar2=-KILLC,
                op0=A.is_lt, op1=A.mult,
            )
            nc.vector.tensor_add(xm[:, :], x[:, :], kbias[:, :])
            # probs = exp((xm - M)/T - ln Zf) in one ScalarE pass; dropped
            # entries underflow to 0; DVE runs the token argmax in parallel
            nc.scalar.activation(
                y[:, :], xm[:, :], Exp, bias=bias2[:, :], scale=INV_TEMP
            )
            nc.sync.dma_start(out=probs_e[:, EOS : EOS + 1], in_=y[0:ROWS, 0:1])
            RH = ROWS // 2
            for q in range(2):
                r0, r1 = q * RH, (q + 1) * RH
                nc.sync.dma_start(
                    out=probs_e[r0:r1, AUDIO_START : AUDIO_START + H - 1],
                    in_=y[r0:r1, 1:H],
                )
                nc.sync.dma_start(
                    out=probs_e[r0:r1, AUDIO_START + H - 1 : V],
                    in_=y[ROWS + r0 : ROWS + r1, :],
                )

            nc.vector.tensor_add(yqm[:, :], yq[:, :], kbias[:, :])
            nc.vector.max(out=tm8[:, :], in_=yqm[:, :])
            nc.vector.max_index(out=tidx[:, :], in_max=tm8[:, :], in_values=yqm[:, :])

            # winner across halves (B strictly greater -> B, ties -> A)
            nc.sync.dma_start(out=tvb[:, :], in_=tm8[ROWS:P, 0:1])
            nc.sync.dma_start(out=tib[:, :], in_=tidx[ROWS:P, 0:1])
            nc.vector.tensor_tensor(
                out=wbf[:, :], in0=tvb[:, :], in1=tm8[0:ROWS, 0:1], op=A.is_gt
            )
            nc.vector.tensor_copy(tfa[:, :], tidx[0:ROWS, 0:1])
            nc.vector.tensor_copy(tfb[:, :], tib[:, :])
            nc.vector.tensor_scalar_add(tfb[:, :], tfb[:, :], float(H))
            nc.vector.tensor_sub(td[:, :], tfb[:, :], tfa[:, :])
            nc.vector.tensor_mul(td[:, :], td[:, :], wbf[:, :])
            nc.vector.tensor_add(tokf[:, :], tfa[:, :], td[:, :])

            nc.sync.dma_start(out=tok_e[:, :], in_=tokf[:, :])

    _split_multi_waits(nc)
    return nc


def fast_path_ok(live: np.ndarray) -> bool:
    """True iff per-chunk top-8 pooling recovers every row's exact top-64,
    i.e. no 64-wide chunk of either half holds more than 8 values >= the
    row's 64th-largest (ties counted conservatively)."""
    x = (live[:, 0] * np.float32(2.0)) - live[:, 1]
    tau = np.partition(x, L - TOP_K, axis=1)[:, L - TOP_K]
    ge = x >= tau[:, None]
    if not np.all(ge.sum(axis=1) == TOP_K):
        return False  # ties at the boundary: let the safe path handle them
    for half in range(2):
        g = ge[:, half * H : (half + 1) * H]
        for a, b in CHUNKS:
            if g[:, a:b].sum(axis=1).max() > 8:
                return False
    return True


def _get_nc(fast: bool) -> bass.Bass:
    key = "nc_fast" if fast else "nc_safe"
    if key not in _CACHE:
        _CACHE[key] = _build_nc_fast() if fast else _build_nc_safe()
    return _CACHE[key]


def _gumbel_live_scaled() -> np.ndarray:
    """temp * gumbel noise of jax.random.categorical at the live columns,
    bit-exact vs the reference (jax threefry on CPU), [T, L] float32."""
    if "gs" in _CACHE:
        return _CACHE["gs"]
    import jax
    import jax.numpy as jnp

    with jax.default_device(jax.devices("cpu")[0]):
        keys = jax.random.split(jax.random.key(1), T)
        gum = jax.jit(
            jax.vmap(lambda k: jax.random.gumbel(k, (V,), jnp.float32))
        )(keys)
        gum = np.asarray(gum)
    gl = np.empty((T, L), np.float32)
    gl[:, 0] = gum[:, EOS]
    gl[:, 1:] = gum[:, AUDIO_START:]
    gl *= np.float32(TEMPERATURE)
    _CACHE["gs"] = gl
    return gl


def make_live(logits: np.ndarray) -> np.ndarray:
    logits = np.asarray(logits, dtype=np.float32)
    live = np.empty((T, 2, L), np.float32)
    live[:, :, 0] = logits[:, :, EOS]
    live[:, :, 1:] = logits[:, :, AUDIO_START:]
    # EOS kill for steps <= MIN_TOKENS: force cond so 2*cond - uncond ~ -1e30,
    # far below any live logit -> never in the top-64, prob exactly 0.
    live[: MIN_TOKENS + 1, 0, 0] = EOS_KILL / 2
    return live


def make_in_maps(live: np.ndarray, fast: bool) -> list[dict[str, np.ndarray]]:
    gs = _gumbel_live_scaled()
    maps = []
    for c in range(N_CORES):
        r0, r1 = c * ROWS, (c + 1) * ROWS
        # partition p<64: row p's live cols [0:H); p>=64: row p-64's [H:L)
        xin2 = np.empty((2 * ROWS, 2 * H), np.float32)
        xin2[:ROWS, 0:H] = live[r0:r1, 0, 0:H]
        xin2[:ROWS, H:] = live[r0:r1, 1, 0:H]
        xin2[ROWS:, 0:H] = live[r0:r1, 0, H:L]
        xin2[ROWS:, H:] = live[r0:r1, 1, H:L]
        gs2 = np.empty((2 * ROWS, H), np.float32)
        gs2[:ROWS] = gs[r0:r1, 0:H]
        gs2[ROWS:] = gs[r0:r1, H:L]
        if fast:
            # stripe-contiguous packing mirroring the fast builder's reads
            blocks = []
            for a, b in STRIPES:
                blocks.append(xin2[:, a:b].ravel())
                blocks.append(xin2[:, H + a : H + b].ravel())
            xinp = np.concatenate(blocks)
            gw = H // 2
            gsp = np.concatenate(
                [gs2[:, 0:gw].ravel(), gs2[:, gw:H].ravel()]
            )
            maps.append({"xin": xinp, "gs": gsp})
        else:
            maps.append({"xin": xin2, "gs": gs2})
    return maps


def postprocess(results: list[dict[str, np.ndarray]]):
    probs = np.concatenate([r["probs"] for r in results], axis=0)
    tidx = np.concatenate(
        [r["tokens"][:, 0].astype(np.int64) for r in results], axis=0
    )
    tokens = np.where(tidx == 0, EOS, AUDIO_START - 1 + tidx).astype(np.int32)
    return tokens, probs


def kernel(logits: np.ndarray):
    live = make_live(logits)
    fast = os.environ.get("SAMPLER_FORCE_PATH", "")
    use_fast = fast_path_ok(live) if fast == "" else (fast == "fast")
    nc = _get_nc(use_fast)
    in_maps = make_in_maps(live, use_fast)
    res = bass_utils.run_bass_kernel_spmd(
        nc, in_maps, core_ids=list(range(N_CORES))
    )
    _CACHE["last_run"] = res
    return postprocess(res.results)


# revision 44
# speedup vs baseline: 1.5762x; 1.0036x over previous
"""Trainium2 Bass kernel for nn_ACE15TEModel_41824391528638 (CFG sampling).

Reference semantics per decode step t over vocab V=155776:
  cfg = uncond + 2.0*(cond - uncond)
  mask tokens < AUDIO_START to -inf (EOS restored when t > MIN_TOKENS)
  top-k(64) filter, top-p(0.9) nucleus filter, temperature 0.85
  probs = softmax(filtered); token = categorical(key_t, filtered) [Gumbel argmax]

Structure exploited:
  * Only the L = V-AUDIO_START+1 = 4108 "live" columns (EOS + audio tokens)
    can survive the audio mask; every other prob column is exactly 0 and no
    dead input column is ever read.  The runner pre-zeros ExternalOutput
    buffers (native path zero-fills, PJRT path donates zero buffers), so the
    kernel writes only the live columns of the full [T, V] probs output.
  * The final keep set is {v : v >= sigma}, sigma derived from the exact
    sorted top-64 values per row via the exp-cumsum crossing of top-p.
  * The Gumbel noise of jax.random.categorical is input-independent ->
    precomputed bit-exactly on host (jax CPU threefry) as a kernel input.

Sharding: steps (T=512) data-parallel over 8 cores; per core 64 steps, each
row split into two 2054-wide halves on SBUF partitions r and r+64.

Fast path (used whenever the host check passes, incl. the spec's input):
  striped DMA + cfg -> per-96-col-chunk top-8 pools (DVE max8) -> cross-
  partition pool merge -> exact row top-64 via 8 rounds of max8+match_replace
  on the 336-wide pool -> top-p cutoff sigma -> additive -512 kill bias ->
  probs = exp((x-M)/T - lnZf) on ScalarE (stores chase only ACT) while DVE
  runs the Gumbel argmax (max8 + max_index) -> merged winner across halves.
  Exactness condition (no chunk holds >8 of its row's top-64) is verified on
  host against the actual input; otherwise the safe builder (full-width
  8-round extraction per half) is selected, which is exact unconditionally.
"""

import os
import sys

import numpy as np

for _p in (
    "/root/.axon_site",
    "/root/.axon_site/_ro/trn_rl_repo",
    "/root/.axon_site/_ro/pypackages",
    "/opt/trn_rl_repo",
):
    if os.path.isdir(_p) and _p not in sys.path:
        sys.path.append(_p)

import concourse.bass as bass
import concourse.mybir as mybir
from concourse import bass_utils
from concourse.tile import TileContext

# Problem constants (hardcoded per spec)
T = 512
V = 155776
AUDIO_START = 151669
EOS = 151645
MIN_TOKENS = 1
CFG_SCALE = 2.0
TEMPERATURE = 0.85
TOP_P = 0.9
TOP_K = 64

N_CORES = 8
ROWS = T // N_CORES            # 64 steps per core
L = V - AUDIO_START + 1        # 4108 live columns: [EOS, AUDIO_START..V)
H = L // 2                     # 2054: row half width (two partitions per row)

NEG = -3.4028235e38            # float32 min (extraction sentinel)
EOS_KILL = -1.0e30             # finite stand-in for -inf EOS mask (avoids inf/nan)
BIG = 3.0e38
KILLC = 512.0                  # additive keep-mask bias: power of two, exact for
                               # {0,1} masks; exp((x-512-M)/temp) underflows to 0
INV_TEMP = 1.0 / TEMPERATURE
KB_C = KILLC * INV_TEMP        # folded into the exp bias so kept entries cancel
# 96-wide chunks over each half row (last takes the 134-col remainder).  The
# fast path pools each chunk's top-8; exact iff no chunk holds >8 of the
# row's top-64 (host-checked per input, safe builder otherwise).
CHUNKS = [(c * 96, (c + 1) * 96) for c in range(20)] + [(1920, H)]
# DMA/compute stripes (in chunks): fat stripes keep DMA descriptors >=2.6KB
# (descriptor rate, not bandwidth, limits thin transfers)
STRIPE_CHUNKS = [(0, 7), (7, 14), (14, 21)]
STRIPES = [(CHUNKS[lo][0], CHUNKS[hi - 1][1]) for lo, hi in STRIPE_CHUNKS]

F32 = mybir.dt.float32
U32 = mybir.dt.uint32

_CACHE: dict = {}


def _ensure_trace_hooks() -> None:
    """If the environment enables BASS_TRACE, run_bass_kernel_spmd imports
    antenv.axon_hooks, which some images lack; synthesize a compatible module
    (wired to the axon NTFF ctypes hook when available) so tracing works
    instead of crashing.  No-op when tracing is off or the module exists."""
    try:
        import antenv.axon_hooks  # noqa: F401
        return
    except ImportError:
        pass
    import types

    try:
        import antenv
    except ImportError:
        return
    mod = types.ModuleType("antenv.axon_hooks")
    holder: dict = {"hook": None, "tried": False}

    def set_axon_ntff_profile_hook(h):
        holder["hook"] = h

    def get_axon_ntff_profile_hook():
        if holder["hook"] is None and not holder["tried"]:
            holder["tried"] = True
            try:
                from trn_agent_boot.trn_boot import _ntff_profile_via_ctypes

                so = "/opt/axon/libaxon_pjrt.so"
                if os.path.exists(so):
                    holder["hook"] = _ntff_profile_via_ctypes(so)
            except Exception:
                pass
        return holder["hook"]

    mod.set_axon_ntff_profile_hook = set_axon_ntff_profile_hook
    mod.get_axon_ntff_profile_hook = get_axon_ntff_profile_hook
    sys.modules["antenv.axon_hooks"] = mod
    antenv.axon_hooks = mod


_ensure_trace_hooks()

# Artifact upload has no remote store in sandboxed runs; degrade locally.
_orig_upload = bass_utils.upload_artifacts


def _safe_upload(tmpdir: str) -> str:
    try:
        return _orig_upload(tmpdir)
    except Exception:
        return tmpdir


bass_utils.upload_artifacts = _safe_upload


def _split_multi_waits(nc: bass.Bass) -> None:
    """This walrus build allows one semaphore wait per compute instruction.
    Hoist extra on_wait entries into standalone InstEventSemaphore ops placed
    just before the instruction on the same engine (engines are in-order, so
    semantics are identical)."""
    skip = (mybir.InstEventSemaphore,)
    for fn in nc.m.functions:
        for blk in fn.blocks:
            new_insts = []
            for inst in blk.instructions:
                si = getattr(inst, "sync_info", None)
                if (
                    si is not None
                    and si.on_wait is not None
                    and len(si.on_wait) > 1
                    and not isinstance(inst, skip)
                ):
                    for w in si.on_wait[:-1]:
                        ev = mybir.InstEventSemaphore(
                            name=f"{inst.name}-wsplit-{w.id}",
                            ins=[],
                            outs=[],
                            sync_info=mybir.SyncInfo(on_wait=[w], on_update=[]),
                        )
                        ev.engine = inst.engine
                        new_insts.append(ev)
                    inst.sync_info = mybir.SyncInfo(
                        on_wait=[si.on_wait[-1]], on_update=si.on_update
                    )
                new_insts.append(inst)
            blk.instructions = new_insts


def _build_nc_safe() -> bass.Bass:
    """128-partition layout: step row r occupies partitions r (live cols
    [0:H), i.e. EOS + audio[0:H-1)) and r+64 (live cols [H:L)).  Per-half
    exact top-64 extraction, then the two halves' candidates are merged into
    both partition groups (redundant small-op compute beats cross-partition
    broadcast latency)."""
    nc = bass.Bass()
    xin_e = nc.declare_dram_parameter("xin", [2 * ROWS, 2 * H], F32, isOutput=False)
    gs_e = nc.declare_dram_parameter("gs", [2 * ROWS, H], F32, isOutput=False)
    probs_e = nc.declare_dram_parameter("probs", [ROWS, V], F32, isOutput=True)
    tok_e = nc.declare_dram_parameter("tokens", [ROWS, 1], F32, isOutput=True)

    K = TOP_K
    P = 2 * ROWS  # 128 partitions
    A = mybir.AluOpType
    X = mybir.AxisListType.X
    Exp = mybir.ActivationFunctionType.Exp
    Copy = mybir.ActivationFunctionType.Copy

    with TileContext(nc) as tc:
        with tc.tile_pool(name="pool", bufs=1) as pool:
            xin = pool.tile([P, 2 * H], F32)
            gs = pool.tile([P, H], F32)
            x = pool.tile([P, H], F32)
            w = pool.tile([P, H], F32)
            mask = pool.tile([P, H], F32)
            xm = pool.tile([P, H], F32)
            y = pool.tile([P, H], F32)
            yq = pool.tile([P, H], F32)
            kbias = pool.tile([P, H], F32)
            yqm = pool.tile([P, H], F32)

            s2 = pool.tile([P, K], F32)       # per-half top-64, sorted desc
            scand = pool.tile([P, 2 * K], F32)  # both halves' candidates
            w2 = pool.tile([P, 2 * K], F32)
            s = pool.tile([P, K], F32)        # row top-64 (same in both groups)
            e64 = pool.tile([P, K], F32)
            cum = pool.tile([P, K], F32)
            gx = pool.tile([P, K], F32)
            km = pool.tile([P, K], F32)
            t64 = pool.tile([P, K], F32)
            sk = pool.tile([P, K], F32)
            ms = pool.tile([P, K], F32)
            zeros64 = pool.tile([P, K], F32)

            negm = pool.tile([P, 1], F32)
            negmtk = pool.tile([P, 1], F32)
            z64 = pool.tile([P, 1], F32)
            thr = pool.tile([P, 1], F32)
            sig = pool.tile([P, 1], F32)
            zf = pool.tile([P, 1], F32)
            zfo = pool.tile([P, 1], F32)
            zfr = pool.tile([P, 1], F32)
            rzf = pool.tile([P, 1], F32)
            tm8 = pool.tile([P, 8], F32)
            tidx = pool.tile([P, 8], U32)
            tvb = pool.tile([ROWS, 1], F32)
            tib = pool.tile([ROWS, 1], U32)
            tfa = pool.tile([ROWS, 1], F32)
            tfb = pool.tile([ROWS, 1], F32)
            wbf = pool.tile([ROWS, 1], F32)
            td = pool.tile([ROWS, 1], F32)
            tokf = pool.tile([ROWS, 1], F32)

            HH = H // 2
            # Loads: cond/unc quarters on separate queues so x can start early
            nc.sync.dma_start(out=xin[:, 0:HH], in_=xin_e[:, 0:HH])
            nc.sync.dma_start(out=xin[:, H : H + HH], in_=xin_e[:, H : H + HH])
            nc.sync.dma_start(out=xin[:, HH:H], in_=xin_e[:, HH:H])
            nc.sync.dma_start(out=xin[:, H + HH : 2 * H], in_=xin_e[:, H + HH : 2 * H])
            nc.sync.dma_start(out=gs[:, 0:HH], in_=gs_e[:, 0:HH])
            nc.sync.dma_start(out=gs[:, HH:H], in_=gs_e[:, HH:H])

            nc.gpsimd.memset(zeros64[:, :], 0.0)

            # x = 2*cond - uncond, in two column stripes
            for c0, c1 in ((0, HH), (HH, H)):
                nc.vector.scalar_tensor_tensor(
                    out=x[:, c0:c1], in0=xin[:, c0:c1], scalar=float(CFG_SCALE),
                    in1=xin[:, H + c0 : H + c1], op0=A.mult, op1=A.subtract,
                )

            # tokens numerator (off the DVE critical path)
            nc.gpsimd.tensor_add(yq[:, :], x[:, :], gs[:, :])

            # per-half exact top-64: 8 rounds of max8 + match_replace
            nc.vector.max(out=s2[:, 0:8], in_=x[:, :])
            nc.vector.match_replace(
                out=w[:, :], in_to_replace=s2[:, 0:8], in_values=x[:, :],
                imm_value=NEG,
            )
            for r in range(1, K // 8):
                sl = s2[:, r * 8 : (r + 1) * 8]
                nc.vector.max(out=sl, in_=w[:, :])
                nc.vector.match_replace(
                    out=w[:, :], in_to_replace=sl, in_values=w[:, :], imm_value=NEG
                )

            # merge the two halves' top-64 into both partition groups
            nc.vector.tensor_copy(scand[:, 0:K], s2[:, :])
            nc.sync.dma_start(out=scand[0:ROWS, K : 2 * K], in_=s2[ROWS:P, :])
            nc.sync.dma_start(out=scand[ROWS:P, K : 2 * K], in_=s2[0:ROWS, :])

            # row top-64 from the 128 candidates
            nc.vector.max(out=s[:, 0:8], in_=scand[:, :])
            nc.vector.match_replace(
                out=w2[:, :], in_to_replace=s[:, 0:8], in_values=scand[:, :],
                imm_value=NEG,
            )
            for r in range(1, K // 8):
                sl = s[:, r * 8 : (r + 1) * 8]
                nc.vector.max(out=sl, in_=w2[:, :])
                nc.vector.match_replace(
                    out=w2[:, :], in_to_replace=sl, in_values=w2[:, :], imm_value=NEG
                )

            # M = s[:,0];  exp biases
            nc.scalar.mul(negm[:, :], s[:, 0:1], -1.0)
            nc.scalar.activation(
                negmtk[:, :], s[:, 0:1], Copy, bias=-KB_C, scale=-INV_TEMP
            )

            # E = exp(s - M), Z64, exclusive cumsum -> top-p cutoff sigma
            nc.scalar.activation(
                e64[:, :], s[:, :], Exp, bias=negm[:, :], scale=1.0,
                accum_out=z64[:, :],
            )
            nc.vector.tensor_tensor_scan(
                out=cum[:, :], data0=e64[:, :], data1=zeros64[:, :],
                initial=0.0, op0=A.add, op1=A.add,
            )
            nc.vector.tensor_sub(gx[:, :], cum[:, :], e64[:, :])
            nc.vector.tensor_scalar(
                out=thr[:, :], in0=z64[:, :], scalar1=float(TOP_P), scalar2=None,
                op0=A.mult,
            )
            nc.vector.tensor_scalar(
                out=km[:, :], in0=gx[:, :], scalar1=thr[:, :], scalar2=None,
                op0=A.is_le,
            )
            nc.vector.tensor_scalar(
                out=t64[:, :], in0=km[:, :], scalar1=-BIG, scalar2=BIG,
                op0=A.mult, op1=A.add,
            )
            nc.vector.tensor_mul(sk[:, :], s[:, :], km[:, :])
            nc.vector.tensor_add(ms[:, :], sk[:, :], t64[:, :])
            nc.vector.tensor_reduce(sig[:, :], ms[:, :], X, A.min)

            # keep mask;  probs path: xm = x + KILLC*mask, exp underflow kills
            nc.vector.tensor_scalar(
                out=mask[:, :], in0=x[:, :], scalar1=sig[:, :], scalar2=None,
                op0=A.is_ge,
            )
            nc.vector.scalar_tensor_tensor(
                out=xm[:, :], in0=mask[:, :], scalar=KILLC, in1=x[:, :],
                op0=A.mult, op1=A.add,
            )
            nc.scalar.activation(
                y[:, :], xm[:, :], Exp, bias=negmtk[:, :], scale=INV_TEMP,
                accum_out=zf[:, :],
            )
            # Zf = sum over both halves: swap-merge across partition groups
            nc.sync.dma_start(out=zfo[0:ROWS, :], in_=zf[ROWS:P, :])
            nc.sync.dma_start(out=zfo[ROWS:P, :], in_=zf[0:ROWS, :])
            nc.vector.tensor_add(zfr[:, :], zf[:, :], zfo[:, :])
            nc.vector.reciprocal(rzf[:, :], zfr[:, :])
            nc.vector.tensor_scalar_mul(y[:, :], y[:, :], rzf[:, :])

            # tokens: yqm = yq + (mask-1)*KILLC, exact for kept entries
            nc.vector.tensor_scalar(
                out=kbias[:, :], in0=mask[:, :], scalar1=KILLC, scalar2=-KILLC,
                op0=A.mult, op1=A.add,
            )
            nc.vector.tensor_add(yqm[:, :], yq[:, :], kbias[:, :])
            nc.vector.max(out=tm8[:, :], in_=yqm[:, :])
            nc.vector.max_index(out=tidx[:, :], in_max=tm8[:, :], in_values=yqm[:, :])

            # winner across halves (B strictly greater -> B, ties -> A)
            nc.sync.dma_start(out=tvb[:, :], in_=tm8[ROWS:P, 0:1])
            nc.sync.dma_start(out=tib[:, :], in_=tidx[ROWS:P, 0:1])
            nc.vector.tensor_tensor(
                out=wbf[:, :], in0=tvb[:, :], in1=tm8[0:ROWS, 0:1], op=A.is_gt
            )
            nc.vector.tensor_copy(tfa[:, :], tidx[0:ROWS, 0:1])
            nc.vector.tensor_copy(tfb[:, :], tib[:, :])
            nc.vector.tensor_scalar_add(tfb[:, :], tfb[:, :], float(H))
            nc.vector.tensor_sub(td[:, :], tfb[:, :], tfa[:, :])
            nc.vector.tensor_mul(td[:, :], td[:, :], wbf[:, :])
            nc.vector.tensor_add(tokf[:, :], tfa[:, :], td[:, :])

            # Stores: only live prob columns; everything else stays zero
            nc.sync.dma_start(out=probs_e[:, EOS : EOS + 1], in_=y[0:ROWS, 0:1])
            nc.sync.dma_start(
                out=probs_e[:, AUDIO_START : AUDIO_START + H - 1],
                in_=y[0:ROWS, 1:H],
            )
            nc.sync.dma_start(
                out=probs_e[:, AUDIO_START + H - 1 : V], in_=y[ROWS:P, :]
            )
            nc.sync.dma_start(out=tok_e[:, :], in_=tokf[:, :])

    _split_multi_waits(nc)
    return nc


def _build_nc_fast() -> bass.Bass:
    """Fast extraction variant: per-chunk top-8 pooling (values only) replaces
    the 8-round full-width extraction.  Exact when no chunk holds more than 8
    of its row's top-64 -- guaranteed by the host-side input check, which
    otherwise selects the safe builder.  Input DMA and cfg are striped so the
    chunk maxes start while later stripes are still loading; the keep-mask
    exp runs on GpSimd/ScalarE in parallel with the token argmax on DVE."""
    nc = bass.Bass()
    P = 2 * ROWS  # 128 partitions
    xin_e = nc.declare_dram_parameter("xin", [P * 2 * H], F32, isOutput=False)
    gs_e = nc.declare_dram_parameter("gs", [P * H], F32, isOutput=False)
    probs_e = nc.declare_dram_parameter("probs", [ROWS, V], F32, isOutput=True)
    tok_e = nc.declare_dram_parameter("tokens", [ROWS, 1], F32, isOutput=True)

    K = TOP_K
    NCH = len(CHUNKS)             # 21 chunks of the half row
    PW = 8 * NCH                  # 168: pooled candidates per half
    A = mybir.AluOpType
    X = mybir.AxisListType.X
    Exp = mybir.ActivationFunctionType.Exp
    Copy = mybir.ActivationFunctionType.Copy

    with TileContext(nc) as tc:
        with tc.tile_pool(name="pool", bufs=1) as pool:
            xin = pool.tile([P, 2 * H], F32)
            gs = pool.tile([P, H], F32)
            x = pool.tile([P, H], F32)
            xm = pool.tile([P, H], F32)
            y = pool.tile([P, H], F32)
            yq = pool.tile([P, H], F32)
            kbias = pool.tile([P, H], F32)
            yqm = pool.tile([P, H], F32)

            scand = pool.tile([P, 2 * PW], F32)  # both halves' chunk top-8 pools
            w2 = pool.tile([P, 2 * PW], F32)
            s = pool.tile([P, K], F32)           # row top-64, sorted desc
            e64 = pool.tile([P, K], F32)
            cum = pool.tile([P, K], F32)
            gx = pool.tile([P, K], F32)
            km = pool.tile([P, K], F32)
            t64 = pool.tile([P, K], F32)
            sk = pool.tile([P, K], F32)
            ms = pool.tile([P, K], F32)
            zeros64 = pool.tile([P, K], F32)

            negm = pool.tile([P, 1], F32)
            negmtk = pool.tile([P, 1], F32)
            z64 = pool.tile([P, 1], F32)
            thr = pool.tile([P, 1], F32)
            sig = pool.tile([P, 1], F32)
            kb64 = pool.tile([P, K], F32)
            sm2 = pool.tile([P, K], F32)
            et = pool.tile([P, K], F32)
            zf = pool.tile([P, 1], F32)
            lnz = pool.tile([P, 1], F32)
            nlnz = pool.tile([P, 1], F32)
            bias2 = pool.tile([P, 1], F32)
            tm8 = pool.tile([P, 8], F32)
            tidx = pool.tile([P, 8], U32)
            tvb = pool.tile([ROWS, 1], F32)
            tib = pool.tile([ROWS, 1], U32)
            tfa = pool.tile([ROWS, 1], F32)
            tfb = pool.tile([ROWS, 1], F32)
            wbf = pool.tile([ROWS, 1], F32)
            td = pool.tile([ROWS, 1], F32)
            tokf = pool.tile([ROWS, 1], F32)

            # stripe loads from host-packed contiguous blocks (cond+unc pairs
            # first-needed-first): fully-linear DRAM reads, fat descriptors
            off = 0
            for si, (a, b) in enumerate(STRIPES):
                w = b - a
                for col0 in (a, H + a):
                    blk = xin_e[off : off + P * w].rearrange("(p c) -> p c", p=P)
                    nc.sync.dma_start(out=xin[:, col0 : col0 + w], in_=blk)
                    off += P * w
            gw = H // 2
            for gi, col0 in enumerate((0, gw)):
                blk = gs_e[gi * P * gw : (gi + 1) * P * gw].rearrange(
                    "(p c) -> p c", p=P
                )
                nc.gpsimd.dma_start(out=gs[:, col0 : col0 + gw], in_=blk)

            nc.gpsimd.memset(zeros64[:, :], 0.0)

            # cfg + chunk top-8s, stripe by stripe
            for si, (a, b) in enumerate(STRIPES):
                nc.vector.scalar_tensor_tensor(
                    out=x[:, a:b], in0=xin[:, a:b], scalar=float(CFG_SCALE),
                    in1=xin[:, H + a : H + b], op0=A.mult, op1=A.subtract,
                )
                for ci in range(*STRIPE_CHUNKS[si]):
                    ca, cb = CHUNKS[ci]
                    nc.vector.max(
                        out=scand[:, 8 * ci : 8 * ci + 8], in_=x[:, ca:cb]
                    )

            # tokens numerator off the DVE critical path
            nc.gpsimd.tensor_add(yq[:, :], x[:, :], gs[:, :])

            # merge both halves' pools into both partition groups
            nc.sync.dma_start(
                out=scand[0:ROWS, PW : 2 * PW], in_=scand[ROWS:P, 0:PW]
            )
            nc.sync.dma_start(
                out=scand[ROWS:P, PW : 2 * PW], in_=scand[0:ROWS, 0:PW]
            )

            # row top-64 from the 512 pooled candidates
            nc.vector.max(out=s[:, 0:8], in_=scand[:, :])
            nc.vector.match_replace(
                out=w2[:, :], in_to_replace=s[:, 0:8], in_values=scand[:, :],
                imm_value=NEG,
            )
            for r in range(1, K // 8):
                sl = s[:, r * 8 : (r + 1) * 8]
                nc.vector.max(out=sl, in_=w2[:, :])
                nc.vector.match_replace(
                    out=w2[:, :], in_to_replace=sl, in_values=w2[:, :], imm_value=NEG
                )

            # M = s[:,0];  exp biases
            nc.scalar.mul(negm[:, :], s[:, 0:1], -1.0)
            nc.scalar.mul(negmtk[:, :], s[:, 0:1], -INV_TEMP)

            # E = exp(s - M), Z64, exclusive cumsum -> top-p cutoff sigma
            nc.scalar.activation(
                e64[:, :], s[:, :], Exp, bias=negm[:, :], scale=1.0,
                accum_out=z64[:, :],
            )
            nc.vector.tensor_tensor_scan(
                out=cum[:, :], data0=e64[:, :], data1=zeros64[:, :],
                initial=0.0, op0=A.add, op1=A.add,
            )
            nc.vector.tensor_sub(gx[:, :], cum[:, :], e64[:, :])
            nc.vector.tensor_scalar(
                out=thr[:, :], in0=z64[:, :], scalar1=float(TOP_P), scalar2=None,
                op0=A.mult,
            )
            nc.vector.tensor_scalar(
                out=km[:, :], in0=gx[:, :], scalar1=thr[:, :], scalar2=None,
                op0=A.is_le,
            )
            nc.vector.tensor_scalar(
                out=t64[:, :], in0=km[:, :], scalar1=-BIG, scalar2=BIG,
                op0=A.mult, op1=A.add,
            )
            nc.vector.tensor_mul(sk[:, :], s[:, :], km[:, :])
            nc.vector.tensor_add(ms[:, :], sk[:, :], t64[:, :])
            nc.vector.tensor_reduce(sig[:, :], ms[:, :], X, A.min)

            # Zf from the sorted top-64 alone: km is the keep mask on s and s
            # is replicated in both partition groups, so no full-width exp and
            # no cross-partition merge are needed at all.
            nc.vector.tensor_scalar(
                out=kb64[:, :], in0=km[:, :], scalar1=KILLC, scalar2=-KILLC,
                op0=A.mult, op1=A.add,
            )
            nc.vector.tensor_add(sm2[:, :], s[:, :], kb64[:, :])
            nc.scalar.activation(
                et[:, :], sm2[:, :], Exp, bias=negmtk[:, :], scale=INV_TEMP,
                accum_out=zf[:, :],
            )
            nc.scalar.activation(
                lnz[:, :], zf[:, :], mybir.ActivationFunctionType.Ln
            )
            nc.scalar.mul(nlnz[:, :], lnz[:, :], -1.0)
            nc.scalar.add(bias2[:, :], negmtk[:, :], nlnz[:, :])

            # fused keep mask -> exact additive kill bias (0 kept, -512 dropped)
            nc.vector.tensor_scalar(
                out=kbias[:, :], in0=x[:, :], scalar1=sig[:, :], scalar2=-KILLC,
                op0=A.is_lt, op1=A.mult,
            )
            nc.vector.tensor_add(xm[:, :], x[:, :], kbias[:, :])
            # probs = exp((xm - M)/T - ln Zf) in one ScalarE pass; dropped
            # entries underflow to 0; DVE runs the token argmax in parallel
            nc.scalar.activation(
                y[:, :], xm[:, :], Exp, bias=bias2[:, :], scale=INV_TEMP
            )
            nc.sync.dma_start(out=probs_e[:, EOS : EOS + 1], in_=y[0:ROWS, 0:1])
            RH = ROWS // 2
            for q in range(2):
                r0, r1 = q * RH, (q + 1) * RH
                nc.sync.dma_start(
                    out=probs_e[r0:r1, AUDIO_START : AUDIO_START + H - 1],
                    in_=y[r0:r1, 1:H],
                )
                nc.sync.dma_start(
                    out=probs_e[r0:r1, AUDIO_START + H - 1 : V],
                    in_=y[ROWS + r0 : ROWS + r1, :],
                )

            nc.vector.tensor_add(yqm[:, :], yq[:, :], kbias[:, :])
            nc.vector.max(out=tm8[:, :], in_=yqm[:, :])
            nc.vector.max_index(out=tidx[:, :], in_max=tm8[:, :], in_values=yqm[:, :])

            # winner across halves (B strictly greater -> B, ties -> A)
            nc.sync.dma_start(out=tvb[:, :], in_=tm8[ROWS:P, 0:1])
            nc.sync.dma_start(out=tib[:, :], in_=tidx[ROWS:P, 0:1])
            nc.vector.tensor_tensor(
                out=wbf[:, :], in0=tvb[:, :], in1=tm8[0:ROWS, 0:1], op=A.is_gt
            )
            nc.vector.tensor_copy(tfa[:, :], tidx[0:ROWS, 0:1])
            nc.vector.tensor_copy(tfb[:, :], tib[:, :])
            nc.vector.tensor_scalar_add(tfb[:, :], tfb[:, :], float(H))
            nc.vector.tensor_sub(td[:, :], tfb[:, :], tfa[:, :])
            nc.vector.tensor_mul(td[:, :], td[:, :], wbf[:, :])
            nc.vector.tensor_add(tokf[:, :], tfa[:, :], td[:, :])

            nc.sync.dma_start(out=tok_e[:, :], in_=tokf[:, :])

    _split_multi_waits(nc)
    return nc


def fast_path_ok(live: np.ndarray) -> bool:
    """True iff per-chunk top-8 pooling recovers every row's exact top-64,
    i.e. no 64-wide chunk of either half holds more than 8 values >= the
    row's 64th-largest (ties counted conservatively)."""
    x = (live[:, 0] * np.float32(2.0)) - live[:, 1]
    tau = np.partition(x, L - TOP_K, axis=1)[:, L - TOP_K]
    ge = x >= tau[:, None]
    if not np.all(ge.sum(axis=1) == TOP_K):
        return False  # ties at the boundary: let the safe path handle them
    for half in range(2):
        g = ge[:, half * H : (half + 1) * H]
        for a, b in CHUNKS:
            if g[:, a:b].sum(axis=1).max() > 8:
                return False
    return True


def _get_nc(fast: bool) -> bass.Bass:
    key = "nc_fast" if fast else "nc_safe"
    if key not in _CACHE:
        _CACHE[key] = _build_nc_fast() if fast else _build_nc_safe()
    return _CACHE[key]


def _gumbel_live_scaled() -> np.ndarray:
    """temp * gumbel noise of jax.random.categorical at the live columns,
    bit-exact vs the reference (jax threefry on CPU), [T, L] float32."""
    if "gs" in _CACHE:
        return _CACHE["gs"]
    import jax
    import jax.numpy as jnp

    with jax.default_device(jax.devices("cpu")[0]):
        keys = jax.random.split(jax.random.key(1), T)
        gum = jax.jit(
            jax.vmap(lambda k: jax.random.gumbel(k, (V,), jnp.float32))
        )(keys)
        gum = np.asarray(gum)
    gl = np.empty((T, L), np.float32)
    gl[:, 0] = gum[:, EOS]
    gl[:, 1:] = gum[:, AUDIO_START:]
    gl *= np.float32(TEMPERATURE)
    _CACHE["gs"] = gl
    return gl


def make_live(logits: np.ndarray) -> np.ndarray:
    logits = np.asarray(logits, dtype=np.float32)
    live = np.empty((T, 2, L), np.float32)
    live[:, :, 0] = logits[:, :, EOS]
    live[:, :, 1:] = logits[:, :, AUDIO_START:]
    # EOS kill for steps <= MIN_TOKENS: force cond so 2*cond - uncond ~ -1e30,
    # far below any live logit -> never in the top-64, prob exactly 0.
    live[: MIN_TOKENS + 1, 0, 0] = EOS_KILL / 2
    return live


def make_in_maps(live: np.ndarray, fast: bool) -> list[dict[str, np.ndarray]]:
    gs = _gumbel_live_scaled()
    maps = []
    for c in range(N_CORES):
        r0, r1 = c * ROWS, (c + 1) * ROWS
        # partition p<64: row p's live cols [0:H); p>=64: row p-64's [H:L)
        xin2 = np.empty((2 * ROWS, 2 * H), np.float32)
        xin2[:ROWS, 0:H] = live[r0:r1, 0, 0:H]
        xin2[:ROWS, H:] = live[r0:r1, 1, 0:H]
        xin2[ROWS:, 0:H] = live[r0:r1, 0, H:L]
        xin2[ROWS:, H:] = live[r0:r1, 1, H:L]
        gs2 = np.empty((2 * ROWS, H), np.float32)
        gs2[:ROWS] = gs[r0:r1, 0:H]
        gs2[ROWS:] = gs[r0:r1, H:L]
        if fast:
            # stripe-contiguous packing mirroring the fast builder's reads
            blocks = []
            for a, b in STRIPES:
                blocks.append(xin2[:, a:b].ravel())
                blocks.append(xin2[:, H + a : H + b].ravel())
            xinp = np.concatenate(blocks)
            gw = H // 2
            gsp = np.concatenate(
                [gs2[:, 0:gw].ravel(), gs2[:, gw:H].ravel()]
            )
            maps.append({"xin": xinp, "gs": gsp})
        else:
            maps.append({"xin": xin2, "gs": gs2})
    return maps


def postprocess(results: list[dict[str, np.ndarray]]):
    probs = np.concatenate([r["probs"] for r in results], axis=0)
    tidx = np.concatenate(
        [r["tokens"][:, 0].astype(np.int64) for r in results], axis=0
    )
    tokens = np.where(tidx == 0, EOS, AUDIO_START - 1 + tidx).astype(np.int32)
    return tokens, probs


def kernel(logits: np.ndarray):
    live = make_live(logits)
    fast = os.environ.get("SAMPLER_FORCE_PATH", "")
    use_fast = fast_path_ok(live) if fast == "" else (fast == "fast")
    nc = _get_nc(use_fast)
    in_maps = make_in_maps(live, use_fast)
    res = bass_utils.run_bass_kernel_spmd(
        nc, in_maps, core_ids=list(range(N_CORES))
    )
    _CACHE["last_run"] = res
    return postprocess(res.results)
